# revision 3
# baseline (speedup 1.0000x reference)
"""Trainium2 Bass kernel for a 3-layer SAGE+GCN GNN on 50k nodes / 800k edges,
sharded across 8 NeuronCores.

Strategy:
  - Nodes are sharded into 8 contiguous ranges; edges assigned to the core that
    owns their dst node (host sorts edges by dst).
  - Per conv, the input features are pre-projected to 128 dims ("table" rows,
    bf16), so every gather moves 256B rows regardless of layer width.
  - Tables for layer 1 are built replicated (each core projects the full x);
    tables for layers 2/3 are built per-shard and exchanged with an AllGather.
  - Aggregation: indirect-DMA gather of each dst-tile's edge rows into SBUF
    (128 edges per partition-chunk), then a one-hot matmul segment-sum on the
    TensorEngine (M[e,d] = (dstlocal[e]==d) * w_e built on VectorE, where w_e
    folds the SAGE mean 1/deg or the GCN edge norm).
  - Everything node-indexed on-chip is kept feature-major ("transposed",
    [feat, node]) so no on-device transposes are ever needed.
"""

import os
import numpy as np
import ml_dtypes

P = 128
NCORES = 8

LAST_EXEC_NS = None
LAST_TRACE = None


# ----------------------------------------------------------------------------
# host-side preprocessing
# ----------------------------------------------------------------------------

REG = int(os.environ.get("GNN_REG", "25000"))  # nodes per dma_gather region (int16 limit)


def _edge_streams(src, dst, w_edge, n, shard, nt):
    """Per-core gather/M-build streams for one edge set, organized for
    nc.gpsimd.dma_gather: per (dst-tile t, src-region r) one gather of
    NV[t][r] valid rows (idx-0 padded to a cross-core-uniform count) plus
    trailing -1 slots up to a chunk multiple.

    Returns per-core (idx16 [128, SIC], dw [128, 2*SK]) plus layout lists.
    """
    nreg = (n + REG - 1) // REG
    percore = []
    counts = np.zeros((NCORES, nt, nreg), np.int64)
    for c in range(NCORES):
        lo, hi = c * shard, (c + 1) * shard
        m = (dst >= lo) & (dst < hi)
        s_c, d_c, w_c = src[m], dst[m] - lo, w_edge[m]
        reg_c = s_c // REG
        order = np.lexsort((reg_c, d_c // P))   # by (tile, region)
        s_c, d_c, w_c, reg_c = s_c[order], d_c[order], w_c[order], reg_c[order]
        key = (d_c // P) * nreg + reg_c
        bounds = np.searchsorted(key, np.arange(nt * nreg + 1))
        counts[c] = (bounds[1:] - bounds[:-1]).reshape(nt, nreg)
        percore.append((s_c, d_c, w_c, bounds))
    NV = counts.max(axis=0)                       # [nt, nreg] max real count
    K = np.maximum((NV + P - 1) // P, 1)          # chunks per (t, r)
    NV = K * P                                    # all slots valid (idx-0 pads)
    Ktot = K.sum(axis=1)                          # chunks per tile
    SK = int(Ktot.sum())
    # column offsets: chunk columns per (t, r); idx16 columns per (t, r)
    coff = np.zeros((nt, nreg), np.int64)
    ioff = np.zeros((nt, nreg), np.int64)
    acc_c = 0
    acc_i = 0
    for t in range(nt):
        for r in range(nreg):
            coff[t, r] = acc_c
            ioff[t, r] = acc_i
            acc_c += K[t, r]
            acc_i += K[t, r] * 8
    SIC = int(acc_i)

    outs = []
    for c in range(NCORES):
        s_c, d_c, w_c, bounds = percore[c]
        idx16 = np.zeros((P, SIC), np.int16)
        dl = np.full((P, SK), -1.0, np.float32)
        wv = np.zeros((P, SK), np.float32)
        for t in range(nt):
            for r in range(nreg):
                b0, b1 = bounds[t * nreg + r], bounds[t * nreg + r + 1]
                cnt = b1 - b0
                kr = int(K[t, r])
                slots = kr * P
                buf_i = np.zeros(slots, np.int32)   # idx-0 pads (always write)
                buf_i[:cnt] = s_c[b0:b1] - r * REG
                buf_d = np.full(slots, -1.0, np.float32)
                buf_d[:cnt] = (d_c[b0:b1] - t * P).astype(np.float32)
                buf_w = np.zeros(slots, np.float32)
                buf_w[:cnt] = w_c[b0:b1]
                # idx16: wrapped over 16 partitions, replicated x8
                cols = kr * 8
                wrap = buf_i.reshape(cols, 16).T.astype(np.int16)  # [16, cols]
                io = int(ioff[t, r])
                idx16[:, io:io + cols] = np.tile(wrap, (8, 1))
                # dl/wv: slot i -> partition i%128, chunk coff+i//128
                co = int(coff[t, r])
                dl[:, co:co + kr] = buf_d.reshape(kr, P).T
                wv[:, co:co + kr] = buf_w.reshape(kr, P).T
        dw = np.concatenate([dl, wv], axis=1).astype(np.float32)
        outs.append((idx16, dw))
    meta_es = dict(K=K.tolist(), NV=NV.tolist(), Ktot=[int(x) for x in Ktot],
                   coff=coff.tolist(), ioff=ioff.tolist(),
                   SK=SK, SIC=SIC, nreg=nreg)
    return outs, meta_es


def _prep(inputs):
    inp = {k: np.asarray(v) for k, v in inputs.items()}
    x = inp["x"].astype(np.float32)
    n, din = x.shape
    assert din == P
    shard = n // NCORES
    nt = (shard + P - 1) // P
    ntx = (n + P - 1) // P
    npad = ntx * P

    src = inp["edge_index"][0].astype(np.int64)
    dst = inp["edge_index"][1].astype(np.int64)
    srca = inp["edge_index_aux"][0].astype(np.int64)
    dsta = inp["edge_index_aux"][1].astype(np.int64)

    deg = np.zeros(n, np.float32)
    np.add.at(deg, dst, 1.0)
    recip_deg = (1.0 / np.maximum(deg, 1.0)).astype(np.float32)
    dega = np.zeros(n, np.float32)
    np.add.at(dega, dsta, 1.0)
    deg_hat = dega + 1.0
    rs = (1.0 / np.sqrt(deg_hat)).astype(np.float32)
    recip_deghat = (1.0 / deg_hat).astype(np.float32)

    sage_streams, es_s = _edge_streams(src, dst, recip_deg[dst], n, shard, nt)
    # GCN self-loop == a (i,i) edge with the same w = rs[dst] form, so fold it
    # into the edge stream (gather row i of the gcn half, scaled by rs[i])
    allnodes = np.arange(n, dtype=np.int64)
    srca_x = np.concatenate([srca, allnodes])
    dsta_x = np.concatenate([dsta, allnodes])
    gcn_streams, es_g = _edge_streams(srca_x, dsta_x, rs[dsta_x], n, shard, nt)

    bf16 = ml_dtypes.bfloat16

    # global transposed x tiles: xt[i*P+f, j] = x[i*P+j, f]
    xpad = np.zeros((npad, P), np.float32)
    xpad[:n] = x
    xt = np.ascontiguousarray(
        xpad.reshape(ntx, P, P).transpose(0, 2, 1).reshape(npad, P)
    ).astype(bf16)

    # packed bf16 weights [P, 2048]
    def w2(a):  # [d,128] -> list of [128,128] tiles
        a = np.asarray(a, np.float32)
        return [a[i * P:(i + 1) * P] for i in range(a.shape[0] // P)]

    wb_tiles = []
    wb_off = {}

    def put_b(name, tiles):
        wb_off[name] = len(wb_tiles) * P
        wb_tiles.extend(tiles)

    put_b("fc1", w2(inp["fc1_W"]))
    for l in (1, 2, 3):
        put_b(f"sWl{l}", w2(inp[f"s{l}_Wl"]))
        put_b(f"gW{l}", w2(inp[f"g{l}_W"]))
        put_b(f"sWr{l}", w2(inp[f"s{l}_Wr"]))
    wb = np.concatenate(wb_tiles, axis=1).astype(bf16)  # [128, 16*128]

    # packed fp32 consts [P, ncols]
    wf_cols = []
    wf_off = {}

    def put_f(name, cols):  # cols: [d] or [d,1] with d multiple-of-128 halves
        a = np.asarray(cols, np.float32).reshape(-1)
        wf_off[name] = len(wf_cols)
        for i in range(a.shape[0] // P):
            wf_cols.append(a[i * P:(i + 1) * P])

    put_f("fc1_b", inp["fc1_b"])
    for l in (1, 2, 3):
        put_f(f"s_bl{l}", inp[f"s{l}_bl"])
        put_f(f"g_b{l}", inp[f"g{l}_b"])
    w_scal = [float(inp[f"w{i}"][0]) for i in range(1, 5)]
    for i in range(1, 5):
        put_f(f"h{i}", inp[f"l{i}_W"].reshape(-1) * w_scal[i - 1])
    wf = np.stack(wf_cols, axis=1).astype(np.float32)  # [128, ncols]
    total_bias = float(sum(float(inp[f"l{i}_b"][0]) * w_scal[i - 1]
                           for i in range(1, 5)))

    iota = np.broadcast_to(np.arange(P, dtype=np.float32), (P, P)).astype(bf16)  # in0 stays bf16
    iota = np.ascontiguousarray(iota)

    # rs for global tiling (gcn table row scale, layer1), pad 1.0
    rs_pad = np.ones(npad, np.float32)
    rs_pad[:n] = rs
    rsg = rs_pad.reshape(ntx, P).T.copy()  # [128, ntx]

    meta = dict(n=n, shard=shard, nt=nt, ntx=ntx, npad=npad,
                es_s=es_s, es_g=es_g,
                wb_off=wb_off, wf_off=wf_off, wf_cols=wf.shape[1],
                total_bias=total_bias)

    in_maps = []
    for c in range(NCORES):
        lo = c * shard
        own = np.zeros((nt * P, P), np.float32)
        nown = min(shard, n - lo)
        ownx = np.zeros((nt * P, P), np.float32)
        ownx[:nown] = x[lo:lo + nown]
        xto = np.ascontiguousarray(
            ownx.reshape(nt, P, P).transpose(0, 2, 1).reshape(nt * P, P)
        ).astype(bf16)
        rso = np.ones(nt * P, np.float32)
        rso[:nown] = rs[lo:lo + nown]
        idx_s, dw_s = sage_streams[c]
        idx_g, dw_g = gcn_streams[c]
        in_maps.append({
            "xt": xt, "xto": xto,
            "idxs": idx_s, "dws": dw_s,
            "idxg": idx_g, "dwg": dw_g,
            "wb": wb, "wf": wf, "iota": iota,
            "rsg": rsg, "rso": rso.reshape(nt, P).T.copy(),
        })
    return meta, in_maps


# ----------------------------------------------------------------------------
# device program
# ----------------------------------------------------------------------------

def _build(meta):
    import concourse.bacc as bacc
    import concourse.bass as bass
    import concourse.mybir as mybir
    import concourse.tile as tile

    dt = mybir.dt
    Alu = mybir.AluOpType
    Act = mybir.ActivationFunctionType

    n, shard, nt, ntx, npad = (meta[k] for k in ("n", "shard", "nt", "ntx", "npad"))
    es_s, es_g = meta["es_s"], meta["es_g"]
    SKs, SKg = es_s["SK"], es_g["SK"]
    SICs, SICg = es_s["SIC"], es_g["SIC"]
    wbo, wfo = meta["wb_off"], meta["wf_off"]

    dbg = bool(int(os.environ.get("GNN_DEBUG", "0")))
    nc = bacc.Bacc("TRN2", target_bir_lowering=False, debug=False,
                   num_devices=NCORES)

    def din(name, shape, dtype):
        return nc.dram_tensor(name, shape, dtype, kind="ExternalInput")

    xt_d = din("xt", [npad, P], dt.bfloat16)
    xto_d = din("xto", [nt * P, P], dt.bfloat16)
    idxs_d = din("idxs", [P, SICs], dt.int16)
    dws_d = din("dws", [P, 2 * SKs], dt.float32)
    idxg_d = din("idxg", [P, SICg], dt.int16)
    dwg_d = din("dwg", [P, 2 * SKg], dt.float32)
    wb_d = din("wb", [P, 16 * P], dt.bfloat16)
    wf_d = din("wf", [P, meta["wf_cols"]], dt.float32)
    iota_d = din("iota", [P, P], dt.bfloat16)
    rsg_d = din("rsg", [P, ntx], dt.float32)
    rso_d = din("rso", [P, nt], dt.float32)
    res_d = nc.dram_tensor("res", [P, nt], dt.float32, kind="ExternalOutput")
    if dbg:
        dbg_tbl1 = nc.dram_tensor("dbg_tbl1", [2 * P, 2 * P], dt.bfloat16,
                                  kind="ExternalOutput")
        dbg_gath = nc.dram_tensor("dbg_gath", [P, es_s["Ktot"][0] * P],
                                  dt.bfloat16, kind="ExternalOutput")
        dbg_agg = nc.dram_tensor("dbg_agg", [P, P], dt.float32,
                                 kind="ExternalOutput")
        dbg_x0 = nc.dram_tensor("dbg_x0", [P, P], dt.float32,
                                kind="ExternalOutput")
        dbg_linr = nc.dram_tensor("dbg_linr", [P, P], dt.float32,
                                  kind="ExternalOutput")

    with tile.TileContext(nc) as tc:
        import contextlib
        _stack = contextlib.ExitStack()
        _ppool = _stack.enter_context(tc.tile_pool(name="persist", bufs=1))
        _dpool = _stack.enter_context(
            tc.tile_pool(name="persistd", bufs=1, space="DRAM"))

        def tc_tile(shape, dtype, space="SBUF", addr_space="Local", name="t"):
            pool = _dpool if space == "DRAM" else _ppool
            return pool.tile(shape, dtype, tag=name, name=name,
                             addr_space=addr_space)

        # --- persistent SBUF ---
        f32, b16 = dt.float32, dt.bfloat16
        x0T = tc_tile([P, nt * P], f32, name="x0T")
        x1aT = tc_tile([P, nt * P], f32, name="x1aT")
        x1bT = tc_tile([P, nt * P], f32, name="x1bT")
        linr = [tc_tile([P, P], f32, name=f"linr{t}") for t in range(nt)]
        resb = tc_tile([P, nt], f32, name="resb")
        wb_s = tc_tile([P, 16 * P], b16, name="wb_s")
        wf_s = tc_tile([P, meta["wf_cols"]], f32, name="wf_s")
        iota_s = tc_tile([P, P], b16, name="iota_s")
        rsg_s = tc_tile([P, ntx], f32, name="rsg_s")
        rso_s = tc_tile([P, nt], f32, name="rso_s")
        dws_s = tc_tile([P, 2 * SKs], f32, name="dws_s")
        dwg_s = tc_tile([P, 2 * SKg], f32, name="dwg_s")

        # --- DRAM tables ---
        tbl1 = tc_tile([npad, 2 * P], b16, space="DRAM", name="tbl1")
        tbl2 = tc_tile([n, 2 * P], b16, space="DRAM", addr_space="Shared",
                       name="tbl2")
        tbl3 = tc_tile([n, 2 * P], b16, space="DRAM", addr_space="Shared",
                       name="tbl3")
        sh2 = tc_tile([shard, 2 * P], b16, space="DRAM", name="sh2")
        sh3 = tc_tile([shard, 2 * P], b16, space="DRAM", name="sh3")

        for t_, d_ in ((wb_s, wb_d), (wf_s, wf_d), (iota_s, iota_d),
                       (rsg_s, rsg_d), (rso_s, rso_d),
                       (dws_s, dws_d), (dwg_s, dwg_d)):
            nc.sync.dma_start(out=t_[:], in_=d_[:])

        with (
            tc.tile_pool(name="xp", bufs=3) as xp,
            tc.tile_pool(name="gp", bufs=3) as gp,
            tc.tile_pool(name="mp", bufs=4) as mp,
            tc.tile_pool(name="op", bufs=4) as op,
            tc.tile_pool(name="bp", bufs=4) as bp,
            tc.tile_pool(name="pp", bufs=2, space="PSUM") as pp,
            tc.tile_pool(name="pq", bufs=4, space="PSUM") as pq,
        ):
            def wbt(name, half=0):  # weight tile [128,128]
                o = wbo[name] + half * P
                return wb_s[:, o:o + P]

            def wfc(name, half=0):  # const col [128,1]
                o = wfo[name] + half
                return wf_s[:, o:o + 1]

            # ---- layer 1: full table (replicated over nodes) ----
            for i in range(ntx):
                xt_t = xp.tile([P, P], b16, tag="xt")
                nc.sync.dma_start(out=xt_t[:], in_=xt_d[i * P:(i + 1) * P, :])
                p1 = pq.tile([P, P], f32, tag="pa")
                nc.tensor.matmul(p1[:], lhsT=wbt("fc1"), rhs=xt_t[:],
                                 start=True, stop=True)
                o1 = bp.tile([P, P], b16, tag="o1")
                nc.scalar.activation(o1[:], p1[:], Act.Relu, bias=wfc("fc1_b"))
                ps = pp.tile([P, P], f32, tag="tbl")
                nc.tensor.matmul(ps[:], lhsT=o1[:], rhs=wbt("sWl1"),
                                 start=True, stop=True)
                pg = pp.tile([P, P], f32, tag="lin")
                nc.tensor.matmul(pg[:], lhsT=o1[:], rhs=wbt("gW1"),
                                 start=True, stop=True)
                tb = bp.tile([P, 2 * P], b16, tag="tb")
                nc.vector.tensor_copy(tb[:, 0:P], ps[:])
                nc.scalar.activation(tb[:, P:2 * P], pg[:], Act.Copy,
                                     scale=rsg_s[:, i:i + 1])
                nc.sync.dma_start(out=tbl1[i * P:(i + 1) * P, :], in_=tb[:])

            # ---- layer 1: own shard (x0T, linr1, gself1, head1) ----
            for t in range(nt):
                sl = slice(t * P, (t + 1) * P)
                xo_t = xp.tile([P, P], b16, tag="xt")
                nc.sync.dma_start(out=xo_t[:], in_=xto_d[t * P:(t + 1) * P, :])
                p1 = pq.tile([P, P], f32, tag="pa")
                nc.tensor.matmul(p1[:], lhsT=wbt("fc1"), rhs=xo_t[:],
                                 start=True, stop=True)
                nc.scalar.activation(x0T[:, sl], p1[:], Act.Relu,
                                     bias=wfc("fc1_b"))
                x0b = bp.tile([P, P], b16, tag="o1")
                nc.vector.tensor_copy(x0b[:], x0T[:, sl])
                plr = pp.tile([P, P], f32, tag="lin")
                nc.tensor.matmul(plr[:], lhsT=wbt("sWr1"), rhs=x0b[:],
                                 start=True, stop=True)
                nc.vector.tensor_scalar(linr[t][:], plr[:], wfc("s_bl1"), None,
                                        op0=Alu.add)
                ph = pp.tile([P, 1], f32, tag="lin")
                nc.tensor.matmul(ph[:], lhsT=x0T[:, sl], rhs=wfc("h1"),
                                 start=True, stop=True)
                nc.vector.tensor_copy(resb[:, t:t + 1], ph[:])

            if dbg:
                nc.sync.dma_start(out=dbg_tbl1[:], in_=tbl1[0:2 * P, :])
                nc.sync.dma_start(out=dbg_x0[:], in_=x0T[:, 0:P])
                nc.sync.dma_start(out=dbg_linr[:], in_=linr[0][:])

            tc.strict_bb_all_engine_barrier()

            # ---- conv layers ----
            def conv_tile(kind, t, tbl, lcur):
                es = es_s if kind == "s" else es_g
                idx_d_ = idxs_d if kind == "s" else idxg_d
                dw = dws_s if kind == "s" else dwg_s
                SK = es["SK"]
                Kt = es["Ktot"][t]
                off = es["coff"][t][0]
                nreg = es["nreg"]
                g = gp.tile([P, Kt * P], b16, tag="gath")
                colofs = 0 if kind == "s" else P
                CAPK = 5  # max chunks (640 rows, HW-proven) per dma_gather
                for r in range(nreg):
                    kr = es["K"][t][r]
                    io = es["ioff"][t][r]
                    co = es["coff"][t][r] - off
                    rlo = r * REG
                    rhi = min(n, (r + 1) * REG)
                    for s in range(0, kr, CAPK):
                        kk = min(CAPK, kr - s)
                        it = mp.tile([P, kk * 8], dt.int16, tag="idxt")
                        nc.sync.dma_start(
                            out=it[:],
                            in_=idx_d_[:, io + s * 8:io + (s + kk) * 8])
                        nc.gpsimd.dma_gather(
                            out_ap=g[:, (co + s) * P:(co + s + kk) * P]
                            .rearrange("p (k e) -> p k e", e=P),
                            in_ap=tbl[rlo:rhi, colofs:colofs + P],
                            idxs_ap=it[:],
                            num_idxs=kk * P,
                            num_idxs_reg=kk * P,
                            elem_size=P,
                            elem_step=2 * P)
                pa = pq.tile([P, P], f32, tag="pa")
                for k in range(Kt):
                    m = mp.tile([P, P], b16, tag="m")
                    nc.vector.tensor_scalar(
                        m[:], iota_s[:],
                        dw[:, off + k:off + k + 1],
                        dw[:, SK + off + k:SK + off + k + 1],
                        op0=Alu.is_equal, op1=Alu.mult)
                    nc.tensor.matmul(pa[:], lhsT=g[:, k * P:(k + 1) * P],
                                     rhs=m[:], start=(k == 0),
                                     stop=(k == Kt - 1))
                if dbg and kind == "s" and t == 0 and lcur == 1:
                    nc.sync.dma_start(out=dbg_gath[:],
                                      in_=g[:, :es_s["Ktot"][0] * P])
                    atmp = op.tile([P, P], f32, tag="atmp")
                    nc.vector.tensor_copy(atmp[:], pa[:])
                    nc.sync.dma_start(out=dbg_agg[:], in_=atmp[:])
                o = op.tile([P, P], f32, tag="c" + kind)
                if kind == "s":
                    nc.vector.tensor_tensor(out=o[:], in0=pa[:],
                                            in1=linr[t][:], op=Alu.add)
                else:
                    nc.vector.tensor_scalar(o[:], pa[:], wfc(f"g_b{lcur}"),
                                            None, op0=Alu.add)
                return o

            for l in (1, 2, 3):
                tbl = (tbl1, tbl2, tbl3)[l - 1]
                sh_next = (sh2, sh3, None)[l - 1]
                tbl_next = (tbl2, tbl3, None)[l - 1]
                for t in range(nt):
                    sl = slice(t * P, (t + 1) * P)
                    oc = conv_tile("s", t, tbl[:], l)
                    oa = conv_tile("g", t, tbl[:], l)
                    if l == 1:
                        nc.vector.tensor_tensor(out=x1aT[:, sl], in0=oc[:],
                                                in1=x0T[:, sl], op=Alu.add)
                        nc.vector.tensor_tensor(out=x1bT[:, sl], in0=oa[:],
                                                in1=x0T[:, sl], op=Alu.add)
                        ocf, oaf = x1aT[:, sl], x1bT[:, sl]
                    else:
                        # += x0 ; += x1 (for out3/out4)
                        nc.vector.tensor_tensor(out=oc[:], in0=oc[:],
                                                in1=x0T[:, sl], op=Alu.add)
                        nc.vector.tensor_tensor(out=oc[:], in0=oc[:],
                                                in1=x1aT[:, sl], op=Alu.add)
                        nc.vector.tensor_tensor(out=oa[:], in0=oa[:],
                                                in1=x0T[:, sl], op=Alu.add)
                        nc.vector.tensor_tensor(out=oa[:], in0=oa[:],
                                                in1=x1bT[:, sl], op=Alu.add)
                        ocf, oaf = oc[:], oa[:]
                    # head on out_{l+1}
                    hname = f"h{l + 1}"
                    ph = pp.tile([P, 1], f32, tag="lin")
                    nc.tensor.matmul(ph[:], lhsT=ocf, rhs=wfc(hname, 0),
                                     start=True, stop=False)
                    nc.tensor.matmul(ph[:], lhsT=oaf, rhs=wfc(hname, 1),
                                     start=False, stop=True)
                    nc.vector.tensor_tensor(out=resb[:, t:t + 1],
                                            in0=resb[:, t:t + 1], in1=ph[:],
                                            op=Alu.add)
                    if l == 3:
                        continue
                    # ---- boundary: tables + linr/gself for layer l+1 ----
                    ocb = bp.tile([P, P], b16, tag="ocb")
                    nc.vector.tensor_copy(ocb[:], ocf)
                    oab = bp.tile([P, P], b16, tag="oab")
                    nc.vector.tensor_copy(oab[:], oaf)
                    ln = l + 1
                    ps = pp.tile([P, P], f32, tag="tbl")
                    nc.tensor.matmul(ps[:], lhsT=ocb[:], rhs=wbt(f"sWl{ln}", 0),
                                     start=True, stop=False)
                    nc.tensor.matmul(ps[:], lhsT=oab[:], rhs=wbt(f"sWl{ln}", 1),
                                     start=False, stop=True)
                    pg = pp.tile([P, P], f32, tag="lin")
                    nc.tensor.matmul(pg[:], lhsT=ocb[:], rhs=wbt(f"gW{ln}", 0),
                                     start=True, stop=False)
                    nc.tensor.matmul(pg[:], lhsT=oab[:], rhs=wbt(f"gW{ln}", 1),
                                     start=False, stop=True)
                    tb = bp.tile([P, 2 * P], b16, tag="tb")
                    nc.vector.tensor_copy(tb[:, 0:P], ps[:])
                    nc.scalar.activation(tb[:, P:2 * P], pg[:], Act.Copy,
                                         scale=rso_s[:, t:t + 1])
                    rt = min(P, shard - t * P)
                    nc.sync.dma_start(out=sh_next[t * P:t * P + rt, :],
                                      in_=tb[:rt, :])
                    plr = pp.tile([P, P], f32, tag="tbl")
                    nc.tensor.matmul(plr[:], lhsT=wbt(f"sWr{ln}", 0), rhs=ocb[:],
                                     start=True, stop=False)
                    nc.tensor.matmul(plr[:], lhsT=wbt(f"sWr{ln}", 1), rhs=oab[:],
                                     start=False, stop=True)
                    nc.vector.tensor_scalar(linr[t][:], plr[:],
                                            wfc(f"s_bl{ln}"), None, op0=Alu.add)
                if l < 3:
                    tc.strict_bb_all_engine_barrier()
                    nc.gpsimd.collective_compute(
                        "AllGather", mybir.AluOpType.bypass,
                        replica_groups=[list(range(NCORES))],
                        ins=[sh_next[:]], outs=[tbl_next[:]])
                    tc.strict_bb_all_engine_barrier()

            # ---- output ----
            nc.vector.tensor_scalar(resb[:], resb[:],
                                    float(meta["total_bias"]), None,
                                    op0=Alu.add)
            nc.sync.dma_start(out=res_d[:], in_=resb[:])
        _stack.close()

    nc.compile()
    return nc


# ----------------------------------------------------------------------------
# entry point
# ----------------------------------------------------------------------------

def _run_and_bench(nc, in_maps, iters):
    """Mirror bass2jax.run_bass_via_pjrt's multi-core path, plus an optional
    pipelined repeat loop to measure marginal per-execution device time."""
    import time
    import jax
    import numpy as np
    from jax.sharding import Mesh, PartitionSpec
    from jax.experimental.shard_map import shard_map
    import concourse.mybir as mybir
    from concourse import bass2jax

    bass2jax.install_neuronx_cc_hook()
    partition_name = (nc.partition_id_tensor.name
                      if nc.partition_id_tensor else None)
    in_names, out_names, out_avals, zero_outs = [], [], [], []
    for alloc in nc.m.functions[0].allocations:
        if not isinstance(alloc, mybir.MemoryLocationSet):
            continue
        name = alloc.memorylocations[0].name
        if alloc.kind == "ExternalInput":
            if name != partition_name:
                in_names.append(name)
        elif alloc.kind == "ExternalOutput":
            shape = tuple(alloc.tensor_shape)
            dtype = mybir.dt.np(alloc.dtype)
            out_names.append(name)
            out_avals.append(jax.core.ShapedArray(shape, dtype))
            zero_outs.append(np.zeros(shape, dtype))
    n_params = len(in_names)
    all_in_names = list(in_names) + out_names
    if partition_name is not None:
        all_in_names.append(partition_name)

    def _body(*args):
        operands = list(args)
        if partition_name is not None:
            operands.append(bass2jax.partition_id_tensor())
        outs = bass2jax._bass_exec_p.bind(
            *operands, out_avals=tuple(out_avals),
            in_names=tuple(all_in_names), out_names=tuple(out_names),
            lowering_input_output_aliases=(),
            sim_require_finite=True, sim_require_nnan=True, nc=nc)
        return tuple(outs)

    devices = jax.devices()[:NCORES]
    mesh = Mesh(np.asarray(devices), ("core",))
    in_specs = (PartitionSpec("core"),) * (n_params + len(out_names))
    out_specs = (PartitionSpec("core"),) * len(out_names)
    sharded = jax.jit(shard_map(_body, mesh=mesh, in_specs=in_specs,
                                out_specs=out_specs, check_rep=False),
                      keep_unused=True)
    concat_in = [
        np.concatenate([np.asarray(in_maps[c][nm]) for c in range(NCORES)], 0)
        for nm in in_names]
    concat_zeros = [np.zeros((NCORES * z.shape[0], *z.shape[1:]), z.dtype)
                    for z in zero_outs]
    out_arrs = sharded(*concat_in, *concat_zeros)
    jax.block_until_ready(out_arrs)

    per_exec_ns = None
    if iters > 0:
        from jax.sharding import NamedSharding
        dev_in = [jax.device_put(a, NamedSharding(mesh, PartitionSpec("core")))
                  for a in concat_in]
        dev_zero = [jax.device_put(z, NamedSharding(mesh, PartitionSpec("core")))
                    for z in concat_zeros]
        r = sharded(*dev_in, *dev_zero)
        jax.block_until_ready(r)
        t1 = time.perf_counter()
        rs = [sharded(*dev_in, *dev_zero) for _ in range(iters)]
        jax.block_until_ready(rs)
        t2 = time.perf_counter()
        per_exec_ns = (t2 - t1) / iters * 1e9

    results = [
        {nm: np.asarray(out_arrs[i]).reshape(NCORES, *out_avals[i].shape)[c]
         for i, nm in enumerate(out_names)}
        for c in range(NCORES)]
    return results, per_exec_ns


def kernel(**inputs):
    global LAST_EXEC_NS, LAST_TRACE

    meta, in_maps = _prep(inputs)
    nc = _build(meta)

    iters = int(os.environ.get("GNN_BENCH", "0"))
    results, per_exec_ns = _run_and_bench(nc, in_maps, iters)
    LAST_EXEC_NS = per_exec_ns
    LAST_TRACE = None

    class _R:
        pass
    res = _R()
    res.results = results

    n, shard, nt = meta["n"], meta["shard"], meta["nt"]
    out = np.empty((n, 1), np.float32)
    for c in range(NCORES):
        r = res.results[c]["res"]  # [128, nt]
        out[c * shard:(c + 1) * shard, 0] = r.T.reshape(-1)[:shard]
    return out



# revision 4
# speedup vs baseline: 1.1190x; 1.1190x over previous
"""Trainium2 Bass kernel v2: 3-layer SAGE+GCN GNN, 50k nodes / 800k edges, 8 cores.

Strategy (v2):
  - Nodes sharded 8 ways; edges assigned to dst core; per-conv tables of
    128-dim pre-projected features (bf16, 256B rows) gathered per edge.
  - One-hot segment-sum matmuls with HOST-PRECOMPUTED M matrices: edges are
    dst-sorted per (tile, region), so each 128-edge chunk touches a narrow
    dst window [lo, hi); M (bf16 [128, W]) is streamed from DRAM, no on-chip
    M construction at all.
  - Gathers grouped ~5 dst-tiles per dma_gather call (both src regions),
    idx data streamed in one small DMA per (group, conv).
  - PSUM does all the summing: bias row matmul (start=True) + lin_r seed
    matmuls + narrow-window chunk matmuls accumulate into one [128,128] tile.
  - Residuals: x01a = x0 + x1a computed once (wide), per-tile finish is a
    single DVE add writing the bf16 layer output directly.
  - Tables for layers 2/3 all-gathered (unsplit, barrier-free data deps);
    layer-1 table built replicated from a feature-major copy of x with
    batched DMAs.
  - Head projections accumulate in a persistent PSUM [128, nt] across layers.
"""

import os
import numpy as np
import ml_dtypes

P = 128
NCORES = 8
REG = 25000  # src-region size (int16 idx limit)
GT = 4       # dst tiles per gather group

LAST_EXEC_NS = None
LAST_TRACE = None

bf16 = ml_dtypes.bfloat16


# ----------------------------------------------------------------------------
# host-side preprocessing
# ----------------------------------------------------------------------------

def _edge_streams(src, dst, w_edge, n, shard, nt, groups):
    """Per-core gather idx + M-matrix streams for one edge set.

    Edges are bucketed per (core, tile, region) and dst-sorted inside each
    bucket, chunked into 128-slot chunks (idx-0 padded).  Chunk count K and
    dst window [lo, hi) per chunk are cross-core uniform (SPMD).

    Returns per-core (idx16 [128, SI], M [128, SMW] bf16) + layout dict.
    """
    nreg = (n + REG - 1) // REG
    assert nreg == 2
    percore = []
    counts = np.zeros((NCORES, nt, nreg), np.int64)
    for c in range(NCORES):
        lo_, hi_ = c * shard, (c + 1) * shard
        m = (dst >= lo_) & (dst < hi_)
        s_c, d_c, w_c = src[m], dst[m] - lo_, w_edge[m]
        reg_c = s_c // REG
        order = np.lexsort((s_c, d_c, reg_c, d_c // P))
        s_c, d_c, w_c, reg_c = s_c[order], d_c[order], w_c[order], reg_c[order]
        key = (d_c // P) * nreg + reg_c
        bounds = np.searchsorted(key, np.arange(nt * nreg + 1))
        counts[c] = (bounds[1:] - bounds[:-1]).reshape(nt, nreg)
        percore.append((s_c, d_c, w_c, bounds))
    K = (counts.max(axis=0) + P - 1) // P  # [nt, nreg] chunks per cell

    # chunk windows: per (t, r, k) union of per-core dst ranges (tile-local)
    nchunks = int(K.sum())
    cid0 = np.zeros((nt, nreg), np.int64)  # first chunk id per cell
    acc = 0
    for t in range(nt):
        for r in range(nreg):
            cid0[t, r] = acc
            acc += K[t, r]
    wlo = np.full(nchunks, P, np.int64)
    whi = np.zeros(nchunks, np.int64)
    for c in range(NCORES):
        s_c, d_c, w_c, bounds = percore[c]
        for t in range(nt):
            for r in range(nreg):
                b0, b1 = bounds[t * nreg + r], bounds[t * nreg + r + 1]
                cnt = b1 - b0
                if cnt == 0:
                    continue
                dl = d_c[b0:b1] - t * P
                kk = (cnt + P - 1) // P
                for k in range(kk):
                    cid = cid0[t, r] + k
                    seg = dl[k * P:(k + 1) * P]
                    wlo[cid] = min(wlo[cid], int(seg.min()))
                    whi[cid] = max(whi[cid], int(seg.max()) + 1)
    wlo = np.minimum(wlo, whi)  # empty chunks (shouldn't happen) -> W=0->1
    W = np.maximum(whi - wlo, 1)
    mco = np.zeros(nchunks + 1, np.int64)
    mco[1:] = np.cumsum(W)
    SMW = int(mco[-1])

    # idx column layout: per (group, region) call, cols = sum K * 8
    ioff = {}
    acc = 0
    for (t0, tc) in groups:
        for r in range(nreg):
            ioff[(t0, r)] = acc
            acc += int(K[t0:t0 + tc, r].sum()) * 8
    SI = acc

    outs = []
    for c in range(NCORES):
        s_c, d_c, w_c, bounds = percore[c]
        idx16 = np.zeros((P, SI), np.int16)
        M = np.zeros((P, SMW), np.float32)
        for (t0, tc) in groups:
            for r in range(nreg):
                call_idx = []
                for t in range(t0, t0 + tc):
                    b0, b1 = bounds[t * nreg + r], bounds[t * nreg + r + 1]
                    cnt = b1 - b0
                    kk = int(K[t, r])
                    slots = kk * P
                    buf_i = np.zeros(slots, np.int32)
                    buf_i[:cnt] = s_c[b0:b1] - r * REG
                    call_idx.append(buf_i)
                    # M fill for this cell
                    if cnt:
                        pos = np.arange(cnt)
                        cids = cid0[t, r] + pos // P
                        cols = mco[cids] + (d_c[b0:b1] - t * P) - wlo[cids]
                        M[pos % P, cols] = w_c[b0:b1]
                if not call_idx:
                    continue
                li = np.concatenate(call_idx)
                cols_n = li.shape[0] // 16
                wrap = li.reshape(cols_n, 16).T.astype(np.int16)
                io = ioff[(t0, r)]
                idx16[:, io:io + cols_n] = np.tile(wrap, (8, 1))
        outs.append((idx16, M.astype(bf16)))
    meta_es = dict(K=K.tolist(), cid0=cid0.tolist(),
                   wlo=wlo.tolist(), W=W.tolist(), mco=mco.tolist(),
                   SMW=SMW, SI=SI, ioff={f"{k[0]}_{k[1]}": v
                                         for k, v in ioff.items()},
                   nreg=nreg)
    return outs, meta_es


def _prep(inputs):
    inp = {k: np.asarray(v) for k, v in inputs.items()}
    x = inp["x"].astype(np.float32)
    n, din = x.shape
    assert din == P
    shard = n // NCORES
    nt = (shard + P - 1) // P
    ntx = (n + P - 1) // P
    npad = ntx * P

    groups = []
    t0 = 0
    while t0 < nt:
        tc = min(GT, nt - t0)
        groups.append((t0, tc))
        t0 += tc

    src = inp["edge_index"][0].astype(np.int64)
    dst = inp["edge_index"][1].astype(np.int64)
    srca = inp["edge_index_aux"][0].astype(np.int64)
    dsta = inp["edge_index_aux"][1].astype(np.int64)

    deg = np.zeros(n, np.float32)
    np.add.at(deg, dst, 1.0)
    recip_deg = (1.0 / np.maximum(deg, 1.0)).astype(np.float32)
    dega = np.zeros(n, np.float32)
    np.add.at(dega, dsta, 1.0)
    deg_hat = dega + 1.0
    rs = (1.0 / np.sqrt(deg_hat)).astype(np.float32)

    es_s_outs, es_s = _edge_streams(src, dst, recip_deg[dst], n, shard, nt,
                                    groups)
    allnodes = np.arange(n, dtype=np.int64)
    srca_x = np.concatenate([srca, allnodes])
    dsta_x = np.concatenate([dsta, allnodes])
    es_g_outs, es_g = _edge_streams(srca_x, dsta_x,
                                    rs[srca_x] * rs[dsta_x], n, shard, nt,
                                    groups)

    # feature-major global x (for replicated layer-1 table build):
    # xtg[f, i*P + j] = x[i*P + j, f]
    xpad = np.zeros((npad, P), np.float32)
    xpad[:n] = x
    xtg = np.ascontiguousarray(xpad.T).astype(bf16)  # [128, npad]

    # packed bf16 weights [P, 16*P]
    def w2(a):
        a = np.asarray(a, np.float32)
        return [a[i * P:(i + 1) * P] for i in range(a.shape[0] // P)]

    wb_tiles = []
    wb_off = {}

    def put_b(name, tiles):
        wb_off[name] = len(wb_tiles) * P
        wb_tiles.extend(tiles)

    put_b("fc1", w2(inp["fc1_W"]))
    for l in (1, 2, 3):
        put_b(f"sWl{l}", w2(inp[f"s{l}_Wl"]))
        put_b(f"gW{l}", w2(inp[f"g{l}_W"]))
        put_b(f"sWr{l}", w2(inp[f"s{l}_Wr"]))
    wb = np.concatenate(wb_tiles, axis=1).astype(bf16)  # [128, 16*128]

    # fp32 consts [128, 1]: fc1 bias col
    wf = np.asarray(inp["fc1_b"], np.float32).reshape(P, 1).copy()

    # bias rows [1, 7*128] bf16: ones, bl1, gb1, bl2, gb2, bl3, gb3
    br_cols = [np.ones(P, np.float32)]
    for l in (1, 2, 3):
        br_cols.append(np.asarray(inp[f"s{l}_bl"], np.float32).reshape(-1))
        br_cols.append(np.asarray(inp[f"g{l}_b"], np.float32).reshape(-1))
    br = np.concatenate(br_cols).reshape(1, -1).astype(bf16)  # [1, 896]

    # head cols [128, 7] bf16 (w_i folded): h1, h2a, h2b, h3a, h3b, h4a, h4b
    w_scal = [float(inp[f"w{i}"][0]) for i in range(1, 5)]
    hcols = [np.asarray(inp["l1_W"], np.float32).reshape(-1) * w_scal[0]]
    for i, l in ((1, 2), (2, 3), (3, 4)):
        hw = np.asarray(inp[f"l{l}_W"], np.float32).reshape(-1) * w_scal[i]
        hcols.append(hw[:P])
        hcols.append(hw[P:])
    hb = np.stack(hcols, axis=1).astype(bf16)  # [128, 7]
    total_bias = float(sum(float(inp[f"l{i}_b"][0]) * w_scal[i - 1]
                           for i in range(1, 5)))

    meta = dict(n=n, shard=shard, nt=nt, ntx=ntx, npad=npad,
                groups=groups, es_s=es_s, es_g=es_g,
                wb_off=wb_off, total_bias=total_bias)

    in_maps = []
    for c in range(NCORES):
        lo = c * shard
        nown = min(shard, n - lo)
        ownx = np.zeros((nt * P, P), np.float32)
        ownx[:nown] = x[lo:lo + nown]
        xto = np.ascontiguousarray(ownx.T).astype(bf16)  # [128, nt*P]
        rso = np.ones(nt * P, np.float32)
        rso[:nown] = rs[lo:lo + nown]
        idx_s, m_s = es_s_outs[c]
        idx_g, m_g = es_g_outs[c]
        in_maps.append({
            "xtg": xtg, "xto": xto,
            "idxs": idx_s, "ms": m_s,
            "idxg": idx_g, "mg": m_g,
            "wb": wb, "wf": wf, "br": br, "hb": hb,
        })
    return meta, in_maps


# ----------------------------------------------------------------------------
# device program
# ----------------------------------------------------------------------------

def _build(meta):
    import contextlib
    import concourse.bacc as bacc
    import concourse.mybir as mybir
    import concourse.tile as tile

    dt = mybir.dt
    Alu = mybir.AluOpType
    Act = mybir.ActivationFunctionType

    n, shard, nt, ntx, npad = (meta[k] for k in ("n", "shard", "nt", "ntx",
                                                 "npad"))
    groups = meta["groups"]
    es = {"s": meta["es_s"], "g": meta["es_g"]}
    wbo = meta["wb_off"]
    f32, b16, f8 = dt.float32, dt.bfloat16, dt.float8e4

    nc = bacc.Bacc("TRN2", target_bir_lowering=False, debug=False,
                   num_devices=NCORES)

    def din(name, shape, dtype):
        return nc.dram_tensor(name, shape, dtype, kind="ExternalInput")

    xtg_d = din("xtg", [P, npad], b16)
    xto_d = din("xto", [P, nt * P], b16)
    idx_d = {"s": din("idxs", [P, es["s"]["SI"]], dt.int16),
             "g": din("idxg", [P, es["g"]["SI"]], dt.int16)}
    m_d = {"s": din("ms", [P, es["s"]["SMW"]], b16),
           "g": din("mg", [P, es["g"]["SMW"]], b16)}
    wb_d = din("wb", [P, 16 * P], b16)
    wf_d = din("wf", [P, 1], f32)
    br_d = din("br", [1, 7 * P], b16)
    hb_d = din("hb", [P, 7], b16)
    res_d = nc.dram_tensor("res", [P, nt], f32, kind="ExternalOutput")

    def cell(kind, t, r):
        e = es[kind]
        return (int(e["K"][t][r]), int(e["cid0"][t][r]))

    with tile.TileContext(nc) as tc:
        _stack = contextlib.ExitStack()
        ppool = _stack.enter_context(tc.tile_pool(name="persist", bufs=1))
        dpool = _stack.enter_context(
            tc.tile_pool(name="persistd", bufs=1, space="DRAM"))

        # --- persistent SBUF ---
        x0T = ppool.tile([P, nt * P], f32, tag="x0T", name="x0T")
        x1aT = ppool.tile([P, nt * P], f32, tag="x1aT", name="x1aT")
        x1bT = ppool.tile([P, nt * P], f32, tag="x1bT", name="x1bT")
        xcb = ppool.tile([P, nt * P], b16, tag="xcb", name="xcb")
        xab = ppool.tile([P, nt * P], b16, tag="xab", name="xab")
        wb_s = ppool.tile([P, 16 * P], b16, tag="wb_s", name="wb_s")
        wf_s = ppool.tile([P, 1], f32, tag="wf_s", name="wf_s")
        br_s = ppool.tile([1, 7 * P], b16, tag="br_s", name="br_s")
        hb_s = ppool.tile([P, 7], b16, tag="hb_s", name="hb_s")
        resb = ppool.tile([P, nt], f32, tag="resb", name="resb")

        # --- DRAM tables (fp8, full 256B rows) ---
        tbl1 = dpool.tile([npad, 2 * P], b16, tag="tbl1", name="tbl1")
        tbl2 = dpool.tile([n, 2 * P], b16, tag="tbl2", name="tbl2",
                          addr_space="Shared")
        tbl3 = dpool.tile([n, 2 * P], b16, tag="tbl3", name="tbl3",
                          addr_space="Shared")
        sh2 = dpool.tile([nt * P, 2 * P], b16, tag="sh2", name="sh2")
        sh3 = dpool.tile([nt * P, 2 * P], b16, tag="sh3", name="sh3")
        tbls = (tbl1, tbl2, tbl3)

        for t_, d_ in ((wb_s, wb_d), (wf_s, wf_d), (br_s, br_d),
                       (hb_s, hb_d)):
            nc.sync.dma_start(out=t_[:], in_=d_[:])

        nc.vector.memset(resb[:], 0.0)

        def wbt(name, half=0):
            o = wbo[name] + half * P
            return wb_s[:, o:o + P]

        def brow(i):  # bias row [1, 128]
            return br_s[:, i * P:(i + 1) * P]

        BGT = 7  # tiles per xtg load batch in phase 1

        with (
            tc.tile_pool(name="bp", bufs=4) as bp,
            tc.tile_pool(name="tp", bufs=2) as tp,
            tc.tile_pool(name="pp", bufs=2, space="PSUM") as pp,
            tc.tile_pool(name="pq", bufs=4, space="PSUM") as pq,
        ):
            # ---- phase 1: replicated tbl1 build (2-tile psum batches) ----
            with tc.tile_pool(name="xp", bufs=2) as xp:
                alt = 0
                for j0 in range(0, ntx, BGT):
                    bgt = min(BGT, ntx - j0)
                    xt_t = xp.tile([P, bgt * P], b16, tag="xt")
                    nc.sync.dma_start(out=xt_t[:],
                                      in_=xtg_d[:, j0 * P:(j0 + bgt) * P])
                    tb = tp.tile([P, bgt * 2 * P], b16, tag="tb")
                    for jj in range(0, bgt, 2):
                        nb = min(2, bgt - jj)
                        p1 = pq.tile([P, nb * P], f32, tag="pa")
                        for q in range(nb):
                            nc.tensor.matmul(
                                p1[:, q * P:(q + 1) * P], lhsT=wbt("fc1"),
                                rhs=xt_t[:, (jj + q) * P:(jj + q + 1) * P],
                                start=True, stop=True)
                        o1 = bp.tile([P, nb * P], b16, tag="o1")
                        alt += 1
                        if alt % 2 == 0:
                            nc.scalar.activation(o1[:], p1[:], Act.Relu,
                                                 bias=wf_s[:, 0:1])
                        else:
                            nc.vector.tensor_scalar(o1[:], p1[:],
                                                    wf_s[:, 0:1], 0.0,
                                                    op0=Alu.add, op1=Alu.max)
                        pt = pp.tile([P, nb * 2 * P], f32, tag="pt")
                        for q in range(nb):
                            oq = o1[:, q * P:(q + 1) * P]
                            nc.tensor.matmul(
                                pt[:, q * 2 * P:q * 2 * P + P],
                                lhsT=oq, rhs=wbt("sWl1"),
                                start=True, stop=True)
                            nc.tensor.matmul(
                                pt[:, q * 2 * P + P:(q + 1) * 2 * P],
                                lhsT=oq, rhs=wbt("gW1"),
                                start=True, stop=True)
                        dst8 = tb[:, jj * 2 * P:(jj + nb) * 2 * P]
                        if alt % 2 == 0:
                            nc.scalar.activation(dst8, pt[:], Act.Copy)
                        else:
                            nc.vector.tensor_copy(dst8, pt[:])
                    nc.sync.dma_start(
                        out=tbl1[j0 * P:(j0 + bgt) * P, :]
                        .rearrange("(jj n) c -> n jj c", n=P),
                        in_=tb[:])

                # ---- phase 2: own-shard fc1 -> x0T, xcb(=bf16 x0) ----
                xo_t = xp.tile([P, nt * P], b16, tag="xo", bufs=1)
                nc.sync.dma_start(out=xo_t[:], in_=xto_d[:])
                for t in range(0, nt, 2):
                    nb = min(2, nt - t)
                    sl = slice(t * P, (t + nb) * P)
                    p1 = pq.tile([P, nb * P], f32, tag="pa")
                    for q in range(nb):
                        nc.tensor.matmul(
                            p1[:, q * P:(q + 1) * P], lhsT=wbt("fc1"),
                            rhs=xo_t[:, (t + q) * P:(t + q + 1) * P],
                            start=True, stop=True)
                    nc.scalar.activation(x0T[:, sl], p1[:], Act.Relu,
                                         bias=wf_s[:, 0:1])
                    nc.vector.tensor_copy(xcb[:, sl], x0T[:, sl])

            # ---- conv layers ----
            with (
                tc.tile_pool(name="gp", bufs=2) as gp,
                tc.tile_pool(name="mp", bufs=2) as mp,
                tc.tile_pool(name="ip", bufs=2) as ip,
            ):
                for l in (1, 2, 3):
                    tbl = tbls[l - 1]
                    sh_next = (sh2, sh3, None)[l - 1]
                    tbl_next = (tbl2, tbl3, None)[l - 1]
                    ln = l + 1
                    for (t0, tc_) in groups:
                        gt = {}
                        mt = {}
                        for kind in ("s", "g"):
                            e = es[kind]
                            ioffA = e["ioff"][f"{t0}_0"]
                            ioffB = e["ioff"][f"{t0}_1"]
                            KA = sum(cell(kind, t, 0)[0]
                                     for t in range(t0, t0 + tc_))
                            KB = sum(cell(kind, t, 1)[0]
                                     for t in range(t0, t0 + tc_))
                            icols = (KA + KB) * 8
                            it = ip.tile([P, icols], dt.int16, tag="it" + kind)
                            nc.sync.dma_start(
                                out=it[:],
                                in_=idx_d[kind][:, ioffA:ioffA + icols])
                            c0 = cell(kind, t0, 0)[1]
                            m0 = e["mco"][c0]
                            if t0 + tc_ < nt:
                                m1 = e["mco"][cell(kind, t0 + tc_, 0)[1]]
                            else:
                                m1 = e["SMW"]
                            mtile = mp.tile([P, m1 - m0], b16, tag="m" + kind)
                            nc.sync.dma_start(out=mtile[:],
                                              in_=m_d[kind][:, m0:m1])
                            mt[kind] = (mtile, m0)
                            colofs = 0 if kind == "s" else P
                            CAPK = 5  # max chunks (640 rows) per dma_gather
                            for r, KR, io in ((0, KA, ioffA), (1, KB, ioffB)):
                                if KR == 0:
                                    gt[(kind, r)] = None
                                    continue
                                g = gp.tile([P, KR * P], b16,
                                            tag=f"g{r}")
                                rlo = r * REG
                                rhi = min(n, rlo + REG)
                                for s in range(0, KR, CAPK):
                                    kk = min(CAPK, KR - s)
                                    nc.gpsimd.dma_gather(
                                        out_ap=g[:, s * P:(s + kk) * P]
                                        .rearrange("p (k e) -> p k e", e=P),
                                        in_ap=tbl[rlo:rhi,
                                                  colofs:colofs + P],
                                        idxs_ap=it[:, io - ioffA + s * 8:
                                                   io - ioffA + (s + kk) * 8],
                                        num_idxs=kk * P,
                                        num_idxs_reg=kk * P,
                                        elem_size=P,
                                        elem_step=2 * P)
                                gt[(kind, r)] = g

                        tbb = None
                        if l < 3:
                            tbb = tp.tile([P, tc_ * 2 * P], b16, tag="tb")
                        for tt in range(tc_):
                            t = t0 + tt
                            sl = slice(t * P, (t + 1) * P)
                            # head on out_l's first term (x0 head) must read
                            # xcb BEFORE this tile's finish overwrites it
                            ph = pq.tile([P, 1], f32, tag="ph", bufs=1)
                            if l == 1:
                                nc.tensor.matmul(ph[:], lhsT=xcb[:, sl],
                                                 rhs=hb_s[:, 0:1],
                                                 start=True, stop=False)
                            for kind in ("s", "g"):
                                e = es[kind]
                                colofs = 0 if kind == "s" else P
                                pa = pq.tile([P, P], f32, tag="pa")
                                bi = (2 * l - 1) if kind == "s" else (2 * l)
                                nc.tensor.matmul(pa[:], lhsT=brow(bi),
                                                 rhs=brow(0), start=True,
                                                 stop=False)
                                if kind == "s":
                                    if l == 1:
                                        nc.tensor.matmul(
                                            pa[:], lhsT=wbt("sWr1"),
                                            rhs=xcb[:, sl],
                                            start=False, stop=False)
                                    else:
                                        nc.tensor.matmul(
                                            pa[:], lhsT=wbt(f"sWr{l}", 0),
                                            rhs=xcb[:, sl],
                                            start=False, stop=False)
                                        nc.tensor.matmul(
                                            pa[:], lhsT=wbt(f"sWr{l}", 1),
                                            rhs=xab[:, sl],
                                            start=False, stop=False)
                                mtile, m0 = mt[kind]
                                chunks = []
                                for r in range(2):
                                    KR, cid_0 = cell(kind, t, r)
                                    gofs = sum(cell(kind, q, r)[0]
                                               for q in range(t0, t))
                                    for k in range(KR):
                                        chunks.append((r, gofs + k,
                                                       cid_0 + k))
                                for ci, (r, gk, cid) in enumerate(chunks):
                                    g = gt[(kind, r)]
                                    lo_c = e["wlo"][cid]
                                    w_c = e["W"][cid]
                                    mc = e["mco"][cid] - m0
                                    last = ci == len(chunks) - 1
                                    nc.tensor.matmul(
                                        pa[:, lo_c:lo_c + w_c],
                                        lhsT=g[:, gk * P:(gk + 1) * P],
                                        rhs=mtile[:, mc:mc + w_c],
                                        start=False, stop=last)
                                # finish
                                if l == 1:
                                    xT = x1aT if kind == "s" else x1bT
                                    xb = xcb if kind == "s" else xab
                                    nc.vector.tensor_tensor(
                                        out=xT[:, sl], in0=pa[:],
                                        in1=x0T[:, sl], op=Alu.add)
                                    if kind == "s":
                                        nc.vector.tensor_copy(xb[:, sl],
                                                              xT[:, sl])
                                    else:
                                        nc.scalar.activation(xb[:, sl],
                                                             xT[:, sl],
                                                             Act.Copy)
                                else:
                                    x01 = x1aT if kind == "s" else x1bT
                                    xb = xcb if kind == "s" else xab
                                    nc.vector.tensor_tensor(
                                        out=xb[:, sl], in0=pa[:],
                                        in1=x01[:, sl], op=Alu.add)
                            # heads on out_{l+1}
                            hia = 2 * l - 1
                            nc.tensor.matmul(ph[:], lhsT=xcb[:, sl],
                                             rhs=hb_s[:, hia:hia + 1],
                                             start=(l != 1), stop=False)
                            nc.tensor.matmul(ph[:], lhsT=xab[:, sl],
                                             rhs=hb_s[:, hia + 1:hia + 2],
                                             start=False, stop=True)
                            nc.vector.tensor_tensor(
                                out=resb[:, t:t + 1], in0=resb[:, t:t + 1],
                                in1=ph[:], op=Alu.add)
                            # next-layer table rows (batched psum)
                            if l < 3:
                                pt = pp.tile([P, 2 * P], f32, tag="pt")
                                nc.tensor.matmul(pt[:, 0:P],
                                                 lhsT=xcb[:, sl],
                                                 rhs=wbt(f"sWl{ln}", 0),
                                                 start=True, stop=False)
                                nc.tensor.matmul(pt[:, 0:P],
                                                 lhsT=xab[:, sl],
                                                 rhs=wbt(f"sWl{ln}", 1),
                                                 start=False, stop=True)
                                nc.tensor.matmul(pt[:, P:2 * P],
                                                 lhsT=xcb[:, sl],
                                                 rhs=wbt(f"gW{ln}", 0),
                                                 start=True, stop=False)
                                nc.tensor.matmul(pt[:, P:2 * P],
                                                 lhsT=xab[:, sl],
                                                 rhs=wbt(f"gW{ln}", 1),
                                                 start=False, stop=True)
                                dst8 = tbb[:, tt * 2 * P:(tt + 1) * 2 * P]
                                if tt % 2 == 0:
                                    nc.vector.tensor_copy(dst8, pt[:])
                                else:
                                    nc.scalar.activation(dst8, pt[:],
                                                         Act.Copy)
                        if l < 3:
                            nc.scalar.dma_start(
                                out=sh_next[t0 * P:(t0 + tc_) * P, :]
                                .rearrange("(jj n) c -> n jj c", n=P),
                                in_=tbb[:])
                    if l < 3:
                        tc.strict_bb_all_engine_barrier()
                        nc.gpsimd.collective_compute(
                            "AllGather", mybir.AluOpType.bypass,
                            replica_groups=[list(range(NCORES))],
                            ins=[sh_next[0:shard, :]], outs=[tbl_next[:]])
                        tc.strict_bb_all_engine_barrier()
                    if l == 1:
                        nc.vector.tensor_tensor(out=x1aT[:], in0=x1aT[:],
                                                in1=x0T[:], op=Alu.add)
                        nc.vector.tensor_tensor(out=x1bT[:], in0=x1bT[:],
                                                in1=x0T[:], op=Alu.add)

            # ---- output ----
            nc.vector.tensor_scalar(resb[:], resb[:],
                                    float(meta["total_bias"]), None,
                                    op0=Alu.add)
            nc.scalar.dma_start(out=res_d[:], in_=resb[:])
        _stack.close()

    nc.compile()
    return nc


def _run_and_bench(nc, in_maps, iters):
    import time
    import jax
    import numpy as np
    from jax.sharding import Mesh, PartitionSpec
    from jax.experimental.shard_map import shard_map
    import concourse.mybir as mybir
    from concourse import bass2jax

    bass2jax.install_neuronx_cc_hook()
    partition_name = (nc.partition_id_tensor.name
                      if nc.partition_id_tensor else None)
    in_names, out_names, out_avals, zero_outs = [], [], [], []
    for alloc in nc.m.functions[0].allocations:
        if not isinstance(alloc, mybir.MemoryLocationSet):
            continue
        name = alloc.memorylocations[0].name
        if alloc.kind == "ExternalInput":
            if name != partition_name:
                in_names.append(name)
        elif alloc.kind == "ExternalOutput":
            shape = tuple(alloc.tensor_shape)
            dtype = mybir.dt.np(alloc.dtype)
            out_names.append(name)
            out_avals.append(jax.core.ShapedArray(shape, dtype))
            zero_outs.append(np.zeros(shape, dtype))
    n_params = len(in_names)
    all_in_names = list(in_names) + out_names
    if partition_name is not None:
        all_in_names.append(partition_name)

    def _body(*args):
        operands = list(args)
        if partition_name is not None:
            operands.append(bass2jax.partition_id_tensor())
        outs = bass2jax._bass_exec_p.bind(
            *operands, out_avals=tuple(out_avals),
            in_names=tuple(all_in_names), out_names=tuple(out_names),
            lowering_input_output_aliases=(),
            sim_require_finite=True, sim_require_nnan=True, nc=nc)
        return tuple(outs)

    devices = jax.devices()[:NCORES]
    mesh = Mesh(np.asarray(devices), ("core",))
    in_specs = (PartitionSpec("core"),) * (n_params + len(out_names))
    out_specs = (PartitionSpec("core"),) * len(out_names)
    sharded = jax.jit(shard_map(_body, mesh=mesh, in_specs=in_specs,
                                out_specs=out_specs, check_rep=False),
                      keep_unused=True)
    concat_in = [
        np.concatenate([np.asarray(in_maps[c][nm]) for c in range(NCORES)], 0)
        for nm in in_names]
    concat_zeros = [np.zeros((NCORES * z.shape[0], *z.shape[1:]), z.dtype)
                    for z in zero_outs]
    out_arrs = sharded(*concat_in, *concat_zeros)
    jax.block_until_ready(out_arrs)

    per_exec_ns = None
    if iters > 0:
        from jax.sharding import NamedSharding
        dev_in = [jax.device_put(a, NamedSharding(mesh, PartitionSpec("core")))
                  for a in concat_in]
        dev_zero = [jax.device_put(z, NamedSharding(mesh, PartitionSpec("core")))
                    for z in concat_zeros]
        r = sharded(*dev_in, *dev_zero)
        jax.block_until_ready(r)
        t1 = time.perf_counter()
        rs = [sharded(*dev_in, *dev_zero) for _ in range(iters)]
        jax.block_until_ready(rs)
        t2 = time.perf_counter()
        per_exec_ns = (t2 - t1) / iters * 1e9

    results = [
        {nm: np.asarray(out_arrs[i]).reshape(NCORES, *out_avals[i].shape)[c]
         for i, nm in enumerate(out_names)}
        for c in range(NCORES)]
    return results, per_exec_ns


def kernel(**inputs):
    global LAST_EXEC_NS, LAST_TRACE

    meta, in_maps = _prep(inputs)
    nc = _build(meta)

    iters = int(os.environ.get("GNN_BENCH", "0"))
    results, per_exec_ns = _run_and_bench(nc, in_maps, iters)
    LAST_EXEC_NS = per_exec_ns
    LAST_TRACE = None

    n, shard, nt = meta["n"], meta["shard"], meta["nt"]
    out = np.empty((n, 1), np.float32)
    for c in range(NCORES):
        r = results[c]["res"]  # [128, nt]
        out[c * shard:(c + 1) * shard, 0] = r.T.reshape(-1)[:shard]
    return out


# revision 5
# speedup vs baseline: 1.1864x; 1.0602x over previous
"""Trainium2 Bass kernel v2: 3-layer SAGE+GCN GNN, 50k nodes / 800k edges, 8 cores.

Strategy (v2):
  - Nodes sharded 8 ways; edges assigned to dst core; per-conv tables of
    128-dim pre-projected features (bf16, 256B rows) gathered per edge.
  - One-hot segment-sum matmuls with HOST-PRECOMPUTED M matrices: edges are
    dst-sorted per (tile, region), so each 128-edge chunk touches a narrow
    dst window [lo, hi); M (bf16 [128, W]) is streamed from DRAM, no on-chip
    M construction at all.
  - Gathers grouped ~5 dst-tiles per dma_gather call (both src regions),
    idx data streamed in one small DMA per (group, conv).
  - PSUM does all the summing: bias row matmul (start=True) + lin_r seed
    matmuls + narrow-window chunk matmuls accumulate into one [128,128] tile.
  - Residuals: x01a = x0 + x1a computed once (wide), per-tile finish is a
    single DVE add writing the bf16 layer output directly.
  - Tables for layers 2/3 all-gathered (unsplit, barrier-free data deps);
    layer-1 table built replicated from a feature-major copy of x with
    batched DMAs.
  - Head projections accumulate in a persistent PSUM [128, nt] across layers.
"""

import os
import numpy as np
import ml_dtypes

P = 128
NCORES = 8
REG = 25000  # src-region size (int16 idx limit)
GT = 4       # dst tiles per gather group

LAST_EXEC_NS = None
LAST_TRACE = None

bf16 = ml_dtypes.bfloat16


# ----------------------------------------------------------------------------
# host-side preprocessing
# ----------------------------------------------------------------------------

def _edge_streams(src, dst, w_edge, n, shard, nt, groups):
    """Per-core gather idx + M-matrix streams for one edge set.

    Edges are bucketed per (core, tile, region) and dst-sorted inside each
    bucket, chunked into 128-slot chunks (idx-0 padded).  Chunk count K and
    dst window [lo, hi) per chunk are cross-core uniform (SPMD).

    Returns per-core (idx16 [128, SI], M [128, SMW] bf16) + layout dict.
    """
    nreg = (n + REG - 1) // REG
    assert nreg == 2
    percore = []
    counts = np.zeros((NCORES, nt, nreg), np.int64)
    for c in range(NCORES):
        lo_, hi_ = c * shard, (c + 1) * shard
        m = (dst >= lo_) & (dst < hi_)
        s_c, d_c, w_c = src[m], dst[m] - lo_, w_edge[m]
        reg_c = s_c // REG
        order = np.lexsort((s_c, d_c, reg_c, d_c // P))
        s_c, d_c, w_c, reg_c = s_c[order], d_c[order], w_c[order], reg_c[order]
        key = (d_c // P) * nreg + reg_c
        bounds = np.searchsorted(key, np.arange(nt * nreg + 1))
        counts[c] = (bounds[1:] - bounds[:-1]).reshape(nt, nreg)
        percore.append((s_c, d_c, w_c, bounds))
    K = (counts.max(axis=0) + P - 1) // P  # [nt, nreg] chunks per cell

    # chunk windows: per (t, r, k) union of per-core dst ranges (tile-local)
    nchunks = int(K.sum())
    cid0 = np.zeros((nt, nreg), np.int64)  # first chunk id per cell
    acc = 0
    for t in range(nt):
        for r in range(nreg):
            cid0[t, r] = acc
            acc += K[t, r]
    wlo = np.full(nchunks, P, np.int64)
    whi = np.zeros(nchunks, np.int64)
    for c in range(NCORES):
        s_c, d_c, w_c, bounds = percore[c]
        for t in range(nt):
            for r in range(nreg):
                b0, b1 = bounds[t * nreg + r], bounds[t * nreg + r + 1]
                cnt = b1 - b0
                if cnt == 0:
                    continue
                dl = d_c[b0:b1] - t * P
                kk = (cnt + P - 1) // P
                for k in range(kk):
                    cid = cid0[t, r] + k
                    seg = dl[k * P:(k + 1) * P]
                    wlo[cid] = min(wlo[cid], int(seg.min()))
                    whi[cid] = max(whi[cid], int(seg.max()) + 1)
    wlo = np.minimum(wlo, whi)  # empty chunks (shouldn't happen) -> W=0->1
    W = np.maximum(whi - wlo, 1)
    mco = np.zeros(nchunks + 1, np.int64)
    mco[1:] = np.cumsum(W)
    SMW = int(mco[-1])

    # idx column layout: per (group, region) call, cols = sum K * 8
    ioff = {}
    acc = 0
    for (t0, tc) in groups:
        for r in range(nreg):
            ioff[(t0, r)] = acc
            acc += int(K[t0:t0 + tc, r].sum()) * 8
    SI = acc

    outs = []
    for c in range(NCORES):
        s_c, d_c, w_c, bounds = percore[c]
        idx16 = np.zeros((P, SI), np.int16)
        M = np.zeros((P, SMW), np.float32)
        for (t0, tc) in groups:
            for r in range(nreg):
                call_idx = []
                for t in range(t0, t0 + tc):
                    b0, b1 = bounds[t * nreg + r], bounds[t * nreg + r + 1]
                    cnt = b1 - b0
                    kk = int(K[t, r])
                    slots = kk * P
                    buf_i = np.zeros(slots, np.int32)
                    buf_i[:cnt] = s_c[b0:b1] - r * REG
                    call_idx.append(buf_i)
                    # M fill for this cell
                    if cnt:
                        pos = np.arange(cnt)
                        cids = cid0[t, r] + pos // P
                        cols = mco[cids] + (d_c[b0:b1] - t * P) - wlo[cids]
                        M[pos % P, cols] = w_c[b0:b1]
                if not call_idx:
                    continue
                li = np.concatenate(call_idx)
                cols_n = li.shape[0] // 16
                wrap = li.reshape(cols_n, 16).T.astype(np.int16)
                io = ioff[(t0, r)]
                idx16[:, io:io + cols_n] = np.tile(wrap, (8, 1))
        outs.append((idx16, M.astype(bf16)))
    meta_es = dict(K=K.tolist(), cid0=cid0.tolist(),
                   wlo=wlo.tolist(), W=W.tolist(), mco=mco.tolist(),
                   SMW=SMW, SI=SI, ioff={f"{k[0]}_{k[1]}": v
                                         for k, v in ioff.items()},
                   nreg=nreg)
    return outs, meta_es


def _prep(inputs):
    inp = {k: np.asarray(v) for k, v in inputs.items()}
    x = inp["x"].astype(np.float32)
    n, din = x.shape
    assert din == P
    shard = n // NCORES
    nt = (shard + P - 1) // P
    ntx = (n + P - 1) // P
    npad = ntx * P

    groups = []
    t0 = 0
    while t0 < nt:
        tc = min(GT, nt - t0)
        groups.append((t0, tc))
        t0 += tc

    src = inp["edge_index"][0].astype(np.int64)
    dst = inp["edge_index"][1].astype(np.int64)
    srca = inp["edge_index_aux"][0].astype(np.int64)
    dsta = inp["edge_index_aux"][1].astype(np.int64)

    deg = np.zeros(n, np.float32)
    np.add.at(deg, dst, 1.0)
    recip_deg = (1.0 / np.maximum(deg, 1.0)).astype(np.float32)
    dega = np.zeros(n, np.float32)
    np.add.at(dega, dsta, 1.0)
    deg_hat = dega + 1.0
    rs = (1.0 / np.sqrt(deg_hat)).astype(np.float32)

    es_s_outs, es_s = _edge_streams(src, dst, recip_deg[dst], n, shard, nt,
                                    groups)
    allnodes = np.arange(n, dtype=np.int64)
    srca_x = np.concatenate([srca, allnodes])
    dsta_x = np.concatenate([dsta, allnodes])
    es_g_outs, es_g = _edge_streams(srca_x, dsta_x,
                                    rs[srca_x] * rs[dsta_x], n, shard, nt,
                                    groups)

    # feature-major global x (for replicated layer-1 table build):
    # xtg[f, i*P + j] = x[i*P + j, f]
    xpad = np.zeros((npad, P), np.float32)
    xpad[:n] = x
    xtg = np.ascontiguousarray(xpad.T).astype(bf16)  # [128, npad]

    # packed bf16 weights [P, 16*P]
    def w2(a):
        a = np.asarray(a, np.float32)
        return [a[i * P:(i + 1) * P] for i in range(a.shape[0] // P)]

    wb_tiles = []
    wb_off = {}

    def put_b(name, tiles):
        wb_off[name] = len(wb_tiles) * P
        wb_tiles.extend(tiles)

    put_b("fc1", w2(inp["fc1_W"]))
    for l in (1, 2, 3):
        put_b(f"sWl{l}", w2(inp[f"s{l}_Wl"]))
        put_b(f"gW{l}", w2(inp[f"g{l}_W"]))
        put_b(f"sWr{l}", w2(inp[f"s{l}_Wr"]))
    wb = np.concatenate(wb_tiles, axis=1).astype(bf16)  # [128, 16*128]

    # fp32 consts [128, 1]: fc1 bias col
    wf = np.asarray(inp["fc1_b"], np.float32).reshape(P, 1).copy()

    # bias rows [1, 7*128] bf16: ones, bl1, gb1, bl2, gb2, bl3, gb3
    br_cols = [np.ones(P, np.float32)]
    for l in (1, 2, 3):
        br_cols.append(np.asarray(inp[f"s{l}_bl"], np.float32).reshape(-1))
        br_cols.append(np.asarray(inp[f"g{l}_b"], np.float32).reshape(-1))
    br = np.concatenate(br_cols).reshape(1, -1).astype(bf16)  # [1, 896]

    # head cols [128, 7] bf16 (w_i folded): h1, h2a, h2b, h3a, h3b, h4a, h4b
    w_scal = [float(inp[f"w{i}"][0]) for i in range(1, 5)]
    hcols = [np.asarray(inp["l1_W"], np.float32).reshape(-1) * w_scal[0]]
    for i, l in ((1, 2), (2, 3), (3, 4)):
        hw = np.asarray(inp[f"l{l}_W"], np.float32).reshape(-1) * w_scal[i]
        hcols.append(hw[:P])
        hcols.append(hw[P:])
    hb = np.stack(hcols, axis=1).astype(bf16)  # [128, 7]
    total_bias = float(sum(float(inp[f"l{i}_b"][0]) * w_scal[i - 1]
                           for i in range(1, 5)))

    meta = dict(n=n, shard=shard, nt=nt, ntx=ntx, npad=npad,
                groups=groups, es_s=es_s, es_g=es_g,
                wb_off=wb_off, total_bias=total_bias)

    in_maps = []
    for c in range(NCORES):
        lo = c * shard
        nown = min(shard, n - lo)
        ownx = np.zeros((nt * P, P), np.float32)
        ownx[:nown] = x[lo:lo + nown]
        xto = np.ascontiguousarray(ownx.T).astype(bf16)  # [128, nt*P]
        rso = np.ones(nt * P, np.float32)
        rso[:nown] = rs[lo:lo + nown]
        idx_s, m_s = es_s_outs[c]
        idx_g, m_g = es_g_outs[c]
        in_maps.append({
            "xtg": xtg, "xto": xto,
            "idxs": idx_s, "ms": m_s,
            "idxg": idx_g, "mg": m_g,
            "wb": wb, "wf": wf, "br": br, "hb": hb,
        })
    return meta, in_maps


# ----------------------------------------------------------------------------
# device program
# ----------------------------------------------------------------------------

def _build(meta):
    import contextlib
    import concourse.bacc as bacc
    import concourse.mybir as mybir
    import concourse.tile as tile

    dt = mybir.dt
    Alu = mybir.AluOpType
    Act = mybir.ActivationFunctionType

    n, shard, nt, ntx, npad = (meta[k] for k in ("n", "shard", "nt", "ntx",
                                                 "npad"))
    groups = meta["groups"]
    es = {"s": meta["es_s"], "g": meta["es_g"]}
    wbo = meta["wb_off"]
    f32, b16, f8 = dt.float32, dt.bfloat16, dt.float8e4

    nc = bacc.Bacc("TRN2", target_bir_lowering=False, debug=False,
                   num_devices=NCORES)

    def din(name, shape, dtype):
        return nc.dram_tensor(name, shape, dtype, kind="ExternalInput")

    xtg_d = din("xtg", [P, npad], b16)
    xto_d = din("xto", [P, nt * P], b16)
    idx_d = {"s": din("idxs", [P, es["s"]["SI"]], dt.int16),
             "g": din("idxg", [P, es["g"]["SI"]], dt.int16)}
    m_d = {"s": din("ms", [P, es["s"]["SMW"]], b16),
           "g": din("mg", [P, es["g"]["SMW"]], b16)}
    wb_d = din("wb", [P, 16 * P], b16)
    wf_d = din("wf", [P, 1], f32)
    br_d = din("br", [1, 7 * P], b16)
    hb_d = din("hb", [P, 7], b16)
    res_d = nc.dram_tensor("res", [P, nt], f32, kind="ExternalOutput")

    def cell(kind, t, r):
        e = es[kind]
        return (int(e["K"][t][r]), int(e["cid0"][t][r]))

    with tile.TileContext(nc) as tc:
        _stack = contextlib.ExitStack()
        ppool = _stack.enter_context(tc.tile_pool(name="persist", bufs=1))
        dpool = _stack.enter_context(
            tc.tile_pool(name="persistd", bufs=1, space="DRAM"))

        # --- persistent SBUF ---
        x0T = ppool.tile([P, nt * P], f32, tag="x0T", name="x0T")
        x1aT = ppool.tile([P, nt * P], f32, tag="x1aT", name="x1aT")
        x1bT = ppool.tile([P, nt * P], f32, tag="x1bT", name="x1bT")
        xcb = ppool.tile([P, nt * P], b16, tag="xcb", name="xcb")
        xab = ppool.tile([P, nt * P], b16, tag="xab", name="xab")
        wb_s = ppool.tile([P, 16 * P], b16, tag="wb_s", name="wb_s")
        wf_s = ppool.tile([P, 1], f32, tag="wf_s", name="wf_s")
        br_s = ppool.tile([1, 7 * P], b16, tag="br_s", name="br_s")
        hb_s = ppool.tile([P, 7], b16, tag="hb_s", name="hb_s")
        resb = ppool.tile([P, nt], f32, tag="resb", name="resb")

        # --- DRAM tables (fp8, full 256B rows) ---
        tbl1 = dpool.tile([npad, 2 * P], b16, tag="tbl1", name="tbl1")
        tbl2 = dpool.tile([n, 2 * P], b16, tag="tbl2", name="tbl2",
                          addr_space="Shared")
        tbl3 = dpool.tile([n, 2 * P], b16, tag="tbl3", name="tbl3",
                          addr_space="Shared")
        sh2 = dpool.tile([nt * P, 2 * P], b16, tag="sh2", name="sh2")
        sh3 = dpool.tile([nt * P, 2 * P], b16, tag="sh3", name="sh3")
        tbls = (tbl1, tbl2, tbl3)

        for t_, d_ in ((wb_s, wb_d), (wf_s, wf_d), (br_s, br_d),
                       (hb_s, hb_d)):
            nc.sync.dma_start(out=t_[:], in_=d_[:])

        nc.vector.memset(resb[:], 0.0)

        def wbt(name, half=0):
            o = wbo[name] + half * P
            return wb_s[:, o:o + P]

        def brow(i):  # bias row [1, 128]
            return br_s[:, i * P:(i + 1) * P]

        BGT = 7  # tiles per xtg load batch in phase 1

        with (
            tc.tile_pool(name="bp", bufs=4) as bp,
            tc.tile_pool(name="tp", bufs=2) as tp,
            tc.tile_pool(name="pp", bufs=2, space="PSUM") as pp,
            tc.tile_pool(name="pq", bufs=4, space="PSUM") as pq,
        ):
            # ---- phase 1: replicated tbl1 build (2-tile psum batches) ----
            with tc.tile_pool(name="xp", bufs=2) as xp:
                alt = 0
                for j0 in range(0, ntx, BGT):
                    bgt = min(BGT, ntx - j0)
                    xt_t = xp.tile([P, bgt * P], b16, tag="xt")
                    nc.sync.dma_start(out=xt_t[:],
                                      in_=xtg_d[:, j0 * P:(j0 + bgt) * P])
                    tb = tp.tile([P, bgt * 2 * P], b16, tag="tb")
                    for jj in range(0, bgt, 2):
                        nb = min(2, bgt - jj)
                        p1 = pq.tile([P, nb * P], f32, tag="pa")
                        for q in range(nb):
                            nc.tensor.matmul(
                                p1[:, q * P:(q + 1) * P], lhsT=wbt("fc1"),
                                rhs=xt_t[:, (jj + q) * P:(jj + q + 1) * P],
                                start=True, stop=True)
                        o1 = bp.tile([P, nb * P], b16, tag="o1")
                        alt += 1
                        if alt % 2 == 0:
                            nc.scalar.activation(o1[:], p1[:], Act.Relu,
                                                 bias=wf_s[:, 0:1])
                        else:
                            nc.vector.tensor_scalar(o1[:], p1[:],
                                                    wf_s[:, 0:1], 0.0,
                                                    op0=Alu.add, op1=Alu.max)
                        pt = pp.tile([P, nb * 2 * P], f32, tag="pt")
                        for q in range(nb):
                            oq = o1[:, q * P:(q + 1) * P]
                            nc.tensor.matmul(
                                pt[:, q * 2 * P:q * 2 * P + P],
                                lhsT=oq, rhs=wbt("sWl1"),
                                start=True, stop=True)
                            nc.tensor.matmul(
                                pt[:, q * 2 * P + P:(q + 1) * 2 * P],
                                lhsT=oq, rhs=wbt("gW1"),
                                start=True, stop=True)
                        dst8 = tb[:, jj * 2 * P:(jj + nb) * 2 * P]
                        if alt % 2 == 0:
                            nc.scalar.activation(dst8, pt[:], Act.Copy)
                        else:
                            nc.vector.tensor_copy(dst8, pt[:])
                    nc.sync.dma_start(
                        out=tbl1[j0 * P:(j0 + bgt) * P, :]
                        .rearrange("(jj n) c -> n jj c", n=P),
                        in_=tb[:])

                # ---- phase 2: own-shard fc1 -> x0T, xcb(=bf16 x0) ----
                xo_t = xp.tile([P, nt * P], b16, tag="xo", bufs=1)
                nc.sync.dma_start(out=xo_t[:], in_=xto_d[:])
                for t in range(0, nt, 2):
                    nb = min(2, nt - t)
                    sl = slice(t * P, (t + nb) * P)
                    p1 = pq.tile([P, nb * P], f32, tag="pa")
                    for q in range(nb):
                        nc.tensor.matmul(
                            p1[:, q * P:(q + 1) * P], lhsT=wbt("fc1"),
                            rhs=xo_t[:, (t + q) * P:(t + q + 1) * P],
                            start=True, stop=True)
                    nc.scalar.activation(x0T[:, sl], p1[:], Act.Relu,
                                         bias=wf_s[:, 0:1])
                    nc.vector.tensor_copy(xcb[:, sl], x0T[:, sl])

            # ---- conv layers ----
            with (
                tc.tile_pool(name="gp", bufs=2) as gp,
                tc.tile_pool(name="mp", bufs=2) as mp,
                tc.tile_pool(name="ip", bufs=2) as ip,
            ):
                for l in (1, 2, 3):
                    tbl = tbls[l - 1]
                    sh_next = (sh2, sh3, None)[l - 1]
                    tbl_next = (tbl2, tbl3, None)[l - 1]
                    ln = l + 1
                    for (t0, tc_) in groups:
                        gt = {}
                        mt = {}
                        for kind in ("s", "g"):
                            e = es[kind]
                            ioffA = e["ioff"][f"{t0}_0"]
                            ioffB = e["ioff"][f"{t0}_1"]
                            KA = sum(cell(kind, t, 0)[0]
                                     for t in range(t0, t0 + tc_))
                            KB = sum(cell(kind, t, 1)[0]
                                     for t in range(t0, t0 + tc_))
                            icols = (KA + KB) * 8
                            it = ip.tile([P, icols], dt.int16, tag="it" + kind)
                            nc.sync.dma_start(
                                out=it[:],
                                in_=idx_d[kind][:, ioffA:ioffA + icols])
                            c0 = cell(kind, t0, 0)[1]
                            m0 = e["mco"][c0]
                            if t0 + tc_ < nt:
                                m1 = e["mco"][cell(kind, t0 + tc_, 0)[1]]
                            else:
                                m1 = e["SMW"]
                            mtile = mp.tile([P, m1 - m0], b16, tag="m" + kind)
                            nc.sync.dma_start(out=mtile[:],
                                              in_=m_d[kind][:, m0:m1])
                            mt[kind] = (mtile, m0)
                            colofs = 0 if kind == "s" else P
                            CAPK = 5  # max chunks (640 rows) per dma_gather
                            for r, KR, io in ((0, KA, ioffA), (1, KB, ioffB)):
                                if KR == 0:
                                    gt[(kind, r)] = None
                                    continue
                                g = gp.tile([P, KR * P], b16,
                                            tag=f"g{r}")
                                rlo = r * REG
                                rhi = min(n, rlo + REG)
                                for s in range(0, KR, CAPK):
                                    kk = min(CAPK, KR - s)
                                    nc.gpsimd.dma_gather(
                                        out_ap=g[:, s * P:(s + kk) * P]
                                        .rearrange("p (k e) -> p k e", e=P),
                                        in_ap=tbl[rlo:rhi,
                                                  colofs:colofs + P],
                                        idxs_ap=it[:, io - ioffA + s * 8:
                                                   io - ioffA + (s + kk) * 8],
                                        num_idxs=kk * P,
                                        num_idxs_reg=kk * P,
                                        elem_size=P,
                                        elem_step=2 * P)
                                gt[(kind, r)] = g

                        tbb = None
                        if l < 3:
                            tbb = tp.tile([P, tc_ * 2 * P], b16, tag="tb")
                        for tt in range(tc_):
                            t = t0 + tt
                            sl = slice(t * P, (t + 1) * P)
                            # head on out_l's first term (x0 head) must read
                            # xcb BEFORE this tile's finish overwrites it
                            ph = pq.tile([P, 1], f32, tag="ph", bufs=1)
                            if l == 1:
                                nc.tensor.matmul(ph[:], lhsT=xcb[:, sl],
                                                 rhs=hb_s[:, 0:1],
                                                 start=True, stop=False)
                            for kind in ("s", "g"):
                                e = es[kind]
                                colofs = 0 if kind == "s" else P
                                pa = pq.tile([P, P], f32, tag="pa")
                                bi = (2 * l - 1) if kind == "s" else (2 * l)
                                nc.tensor.matmul(pa[:], lhsT=brow(bi),
                                                 rhs=brow(0), start=True,
                                                 stop=False)
                                if kind == "s":
                                    if l == 1:
                                        nc.tensor.matmul(
                                            pa[:], lhsT=wbt("sWr1"),
                                            rhs=xcb[:, sl],
                                            start=False, stop=False)
                                    else:
                                        nc.tensor.matmul(
                                            pa[:], lhsT=wbt(f"sWr{l}", 0),
                                            rhs=xcb[:, sl],
                                            start=False, stop=False)
                                        nc.tensor.matmul(
                                            pa[:], lhsT=wbt(f"sWr{l}", 1),
                                            rhs=xab[:, sl],
                                            start=False, stop=False)
                                mtile, m0 = mt[kind]
                                chunks = []
                                for r in range(2):
                                    KR, cid_0 = cell(kind, t, r)
                                    gofs = sum(cell(kind, q, r)[0]
                                               for q in range(t0, t))
                                    for k in range(KR):
                                        chunks.append((r, gofs + k,
                                                       cid_0 + k))
                                for ci, (r, gk, cid) in enumerate(chunks):
                                    g = gt[(kind, r)]
                                    lo_c = e["wlo"][cid]
                                    w_c = e["W"][cid]
                                    mc = e["mco"][cid] - m0
                                    last = ci == len(chunks) - 1
                                    nc.tensor.matmul(
                                        pa[:, lo_c:lo_c + w_c],
                                        lhsT=g[:, gk * P:(gk + 1) * P],
                                        rhs=mtile[:, mc:mc + w_c],
                                        start=False, stop=last)
                                # finish
                                if l == 1:
                                    xT = x1aT if kind == "s" else x1bT
                                    xb = xcb if kind == "s" else xab
                                    nc.vector.tensor_tensor(
                                        out=xT[:, sl], in0=pa[:],
                                        in1=x0T[:, sl], op=Alu.add)
                                    if kind == "s":
                                        nc.vector.tensor_copy(xb[:, sl],
                                                              xT[:, sl])
                                    else:
                                        nc.scalar.activation(xb[:, sl],
                                                             xT[:, sl],
                                                             Act.Copy)
                                else:
                                    x01 = x1aT if kind == "s" else x1bT
                                    xb = xcb if kind == "s" else xab
                                    nc.vector.tensor_tensor(
                                        out=xb[:, sl], in0=pa[:],
                                        in1=x01[:, sl], op=Alu.add)
                            # heads on out_{l+1}
                            hia = 2 * l - 1
                            nc.tensor.matmul(ph[:], lhsT=xcb[:, sl],
                                             rhs=hb_s[:, hia:hia + 1],
                                             start=(l != 1), stop=False)
                            nc.tensor.matmul(ph[:], lhsT=xab[:, sl],
                                             rhs=hb_s[:, hia + 1:hia + 2],
                                             start=False, stop=True)
                            nc.vector.tensor_tensor(
                                out=resb[:, t:t + 1], in0=resb[:, t:t + 1],
                                in1=ph[:], op=Alu.add)
                            # next-layer table rows (batched psum)
                            if l < 3:
                                pt = pp.tile([P, 2 * P], f32, tag="pt")
                                nc.tensor.matmul(pt[:, 0:P],
                                                 lhsT=xcb[:, sl],
                                                 rhs=wbt(f"sWl{ln}", 0),
                                                 start=True, stop=False)
                                nc.tensor.matmul(pt[:, 0:P],
                                                 lhsT=xab[:, sl],
                                                 rhs=wbt(f"sWl{ln}", 1),
                                                 start=False, stop=True)
                                nc.tensor.matmul(pt[:, P:2 * P],
                                                 lhsT=xcb[:, sl],
                                                 rhs=wbt(f"gW{ln}", 0),
                                                 start=True, stop=False)
                                nc.tensor.matmul(pt[:, P:2 * P],
                                                 lhsT=xab[:, sl],
                                                 rhs=wbt(f"gW{ln}", 1),
                                                 start=False, stop=True)
                                dst8 = tbb[:, tt * 2 * P:(tt + 1) * 2 * P]
                                if tt % 2 == 0:
                                    nc.vector.tensor_copy(dst8, pt[:])
                                else:
                                    nc.scalar.activation(dst8, pt[:],
                                                         Act.Copy)
                        if l < 3:
                            nc.scalar.dma_start(
                                out=sh_next[t0 * P:(t0 + tc_) * P, :]
                                .rearrange("(jj n) c -> n jj c", n=P),
                                in_=tbb[:])
                    if l < 3:
                        nc.gpsimd.collective_compute(
                            "AllGather", mybir.AluOpType.bypass,
                            replica_groups=[list(range(NCORES))],
                            ins=[sh_next[0:shard, :]], outs=[tbl_next[:]])
                    if l == 1:
                        nc.vector.tensor_tensor(out=x1aT[:], in0=x1aT[:],
                                                in1=x0T[:], op=Alu.add)
                        nc.vector.tensor_tensor(out=x1bT[:], in0=x1bT[:],
                                                in1=x0T[:], op=Alu.add)

            # ---- output ----
            nc.vector.tensor_scalar(resb[:], resb[:],
                                    float(meta["total_bias"]), None,
                                    op0=Alu.add)
            nc.scalar.dma_start(out=res_d[:], in_=resb[:])
        _stack.close()

    nc.compile()
    return nc


def _run_and_bench(nc, in_maps, iters):
    import time
    import jax
    import numpy as np
    from jax.sharding import Mesh, PartitionSpec
    from jax.experimental.shard_map import shard_map
    import concourse.mybir as mybir
    from concourse import bass2jax

    bass2jax.install_neuronx_cc_hook()
    partition_name = (nc.partition_id_tensor.name
                      if nc.partition_id_tensor else None)
    in_names, out_names, out_avals, zero_outs = [], [], [], []
    for alloc in nc.m.functions[0].allocations:
        if not isinstance(alloc, mybir.MemoryLocationSet):
            continue
        name = alloc.memorylocations[0].name
        if alloc.kind == "ExternalInput":
            if name != partition_name:
                in_names.append(name)
        elif alloc.kind == "ExternalOutput":
            shape = tuple(alloc.tensor_shape)
            dtype = mybir.dt.np(alloc.dtype)
            out_names.append(name)
            out_avals.append(jax.core.ShapedArray(shape, dtype))
            zero_outs.append(np.zeros(shape, dtype))
    n_params = len(in_names)
    all_in_names = list(in_names) + out_names
    if partition_name is not None:
        all_in_names.append(partition_name)

    def _body(*args):
        operands = list(args)
        if partition_name is not None:
            operands.append(bass2jax.partition_id_tensor())
        outs = bass2jax._bass_exec_p.bind(
            *operands, out_avals=tuple(out_avals),
            in_names=tuple(all_in_names), out_names=tuple(out_names),
            lowering_input_output_aliases=(),
            sim_require_finite=True, sim_require_nnan=True, nc=nc)
        return tuple(outs)

    devices = jax.devices()[:NCORES]
    mesh = Mesh(np.asarray(devices), ("core",))
    in_specs = (PartitionSpec("core"),) * (n_params + len(out_names))
    out_specs = (PartitionSpec("core"),) * len(out_names)
    sharded = jax.jit(shard_map(_body, mesh=mesh, in_specs=in_specs,
                                out_specs=out_specs, check_rep=False),
                      keep_unused=True)
    concat_in = [
        np.concatenate([np.asarray(in_maps[c][nm]) for c in range(NCORES)], 0)
        for nm in in_names]
    concat_zeros = [np.zeros((NCORES * z.shape[0], *z.shape[1:]), z.dtype)
                    for z in zero_outs]
    out_arrs = sharded(*concat_in, *concat_zeros)
    jax.block_until_ready(out_arrs)

    per_exec_ns = None
    if iters > 0:
        from jax.sharding import NamedSharding
        dev_in = [jax.device_put(a, NamedSharding(mesh, PartitionSpec("core")))
                  for a in concat_in]
        dev_zero = [jax.device_put(z, NamedSharding(mesh, PartitionSpec("core")))
                    for z in concat_zeros]
        r = sharded(*dev_in, *dev_zero)
        jax.block_until_ready(r)
        t1 = time.perf_counter()
        rs = [sharded(*dev_in, *dev_zero) for _ in range(iters)]
        jax.block_until_ready(rs)
        t2 = time.perf_counter()
        per_exec_ns = (t2 - t1) / iters * 1e9

    results = [
        {nm: np.asarray(out_arrs[i]).reshape(NCORES, *out_avals[i].shape)[c]
         for i, nm in enumerate(out_names)}
        for c in range(NCORES)]
    return results, per_exec_ns


def kernel(**inputs):
    global LAST_EXEC_NS, LAST_TRACE

    meta, in_maps = _prep(inputs)
    nc = _build(meta)

    iters = int(os.environ.get("GNN_BENCH", "0"))
    results, per_exec_ns = _run_and_bench(nc, in_maps, iters)
    LAST_EXEC_NS = per_exec_ns
    LAST_TRACE = None

    n, shard, nt = meta["n"], meta["shard"], meta["nt"]
    out = np.empty((n, 1), np.float32)
    for c in range(NCORES):
        r = results[c]["res"]  # [128, nt]
        out[c * shard:(c + 1) * shard, 0] = r.T.reshape(-1)[:shard]
    return out


# revision 7
# speedup vs baseline: 1.2049x; 1.0156x over previous
"""Trainium2 Bass kernel v2: 3-layer SAGE+GCN GNN, 50k nodes / 800k edges, 8 cores.

Strategy (v2):
  - Nodes sharded 8 ways; edges assigned to dst core; per-conv tables of
    128-dim pre-projected features (bf16, 256B rows) gathered per edge.
  - One-hot segment-sum matmuls with HOST-PRECOMPUTED M matrices: edges are
    dst-sorted per (tile, region), so each 128-edge chunk touches a narrow
    dst window [lo, hi); M (bf16 [128, W]) is streamed from DRAM, no on-chip
    M construction at all.
  - Gathers grouped ~5 dst-tiles per dma_gather call (both src regions),
    idx data streamed in one small DMA per (group, conv).
  - PSUM does all the summing: bias row matmul (start=True) + lin_r seed
    matmuls + narrow-window chunk matmuls accumulate into one [128,128] tile.
  - Residuals: x01a = x0 + x1a computed once (wide), per-tile finish is a
    single DVE add writing the bf16 layer output directly.
  - Tables for layers 2/3 all-gathered (unsplit, barrier-free data deps);
    layer-1 table built replicated from a feature-major copy of x with
    batched DMAs.
  - Head projections accumulate in a persistent PSUM [128, nt] across layers.
"""

import os
import numpy as np
import ml_dtypes

P = 128
NCORES = 8
REG = 25000  # src-region size (int16 idx limit)
GT = 4       # dst tiles per gather group

LAST_EXEC_NS = None
LAST_TRACE = None

bf16 = ml_dtypes.bfloat16


# ----------------------------------------------------------------------------
# host-side preprocessing
# ----------------------------------------------------------------------------

def _edge_streams(src, dst, w_edge, n, shard, nt, groups):
    """Per-core gather idx + M-matrix streams for one edge set.

    Edges are bucketed per (core, tile, region) and dst-sorted inside each
    bucket, chunked into 128-slot chunks (idx-0 padded).  Chunk count K and
    dst window [lo, hi) per chunk are cross-core uniform (SPMD).

    Returns per-core (idx16 [128, SI], M [128, SMW] bf16) + layout dict.
    """
    nreg = (n + REG - 1) // REG
    assert nreg == 2
    percore = []
    counts = np.zeros((NCORES, nt, nreg), np.int64)
    for c in range(NCORES):
        lo_, hi_ = c * shard, (c + 1) * shard
        m = (dst >= lo_) & (dst < hi_)
        s_c, d_c, w_c = src[m], dst[m] - lo_, w_edge[m]
        reg_c = s_c // REG
        order = np.lexsort((s_c, d_c, reg_c, d_c // P))
        s_c, d_c, w_c, reg_c = s_c[order], d_c[order], w_c[order], reg_c[order]
        key = (d_c // P) * nreg + reg_c
        bounds = np.searchsorted(key, np.arange(nt * nreg + 1))
        counts[c] = (bounds[1:] - bounds[:-1]).reshape(nt, nreg)
        percore.append((s_c, d_c, w_c, bounds))
    K = (counts.max(axis=0) + P - 1) // P  # [nt, nreg] chunks per cell

    # chunk windows: per (t, r, k) union of per-core dst ranges (tile-local)
    nchunks = int(K.sum())
    cid0 = np.zeros((nt, nreg), np.int64)  # first chunk id per cell
    acc = 0
    for t in range(nt):
        for r in range(nreg):
            cid0[t, r] = acc
            acc += K[t, r]
    wlo = np.full(nchunks, P, np.int64)
    whi = np.zeros(nchunks, np.int64)
    for c in range(NCORES):
        s_c, d_c, w_c, bounds = percore[c]
        for t in range(nt):
            for r in range(nreg):
                b0, b1 = bounds[t * nreg + r], bounds[t * nreg + r + 1]
                cnt = b1 - b0
                if cnt == 0:
                    continue
                dl = d_c[b0:b1] - t * P
                kk = (cnt + P - 1) // P
                for k in range(kk):
                    cid = cid0[t, r] + k
                    seg = dl[k * P:(k + 1) * P]
                    wlo[cid] = min(wlo[cid], int(seg.min()))
                    whi[cid] = max(whi[cid], int(seg.max()) + 1)
    wlo = np.minimum(wlo, whi)  # empty chunks (shouldn't happen) -> W=0->1
    W = np.maximum(whi - wlo, 1)
    mco = np.zeros(nchunks + 1, np.int64)
    mco[1:] = np.cumsum(W)
    SMW = int(mco[-1])

    # idx column layout: per (group, region) call, cols = sum K * 8
    ioff = {}
    acc = 0
    for (t0, tc) in groups:
        for r in range(nreg):
            ioff[(t0, r)] = acc
            acc += int(K[t0:t0 + tc, r].sum()) * 8
    SI = acc

    outs = []
    for c in range(NCORES):
        s_c, d_c, w_c, bounds = percore[c]
        idx16 = np.zeros((P, SI), np.int16)
        M = np.zeros((P, SMW), np.float32)
        for (t0, tc) in groups:
            for r in range(nreg):
                call_idx = []
                for t in range(t0, t0 + tc):
                    b0, b1 = bounds[t * nreg + r], bounds[t * nreg + r + 1]
                    cnt = b1 - b0
                    kk = int(K[t, r])
                    slots = kk * P
                    buf_i = np.zeros(slots, np.int32)
                    buf_i[:cnt] = s_c[b0:b1] - r * REG
                    call_idx.append(buf_i)
                    # M fill for this cell
                    if cnt:
                        pos = np.arange(cnt)
                        cids = cid0[t, r] + pos // P
                        cols = mco[cids] + (d_c[b0:b1] - t * P) - wlo[cids]
                        M[pos % P, cols] = w_c[b0:b1]
                if not call_idx:
                    continue
                li = np.concatenate(call_idx)
                cols_n = li.shape[0] // 16
                wrap = li.reshape(cols_n, 16).T.astype(np.int16)
                io = ioff[(t0, r)]
                idx16[:, io:io + cols_n] = np.tile(wrap, (8, 1))
        outs.append((idx16, M.astype(bf16)))
    meta_es = dict(K=K.tolist(), cid0=cid0.tolist(),
                   wlo=wlo.tolist(), W=W.tolist(), mco=mco.tolist(),
                   SMW=SMW, SI=SI, ioff={f"{k[0]}_{k[1]}": v
                                         for k, v in ioff.items()},
                   nreg=nreg)
    return outs, meta_es


def _prep(inputs):
    inp = {k: np.asarray(v) for k, v in inputs.items()}
    x = inp["x"].astype(np.float32)
    n, din = x.shape
    assert din == P
    shard = n // NCORES
    nt = (shard + P - 1) // P
    ntx = (n + P - 1) // P
    npad = ntx * P

    groups = []
    t0 = 0
    while t0 < nt:
        tc = min(GT, nt - t0)
        groups.append((t0, tc))
        t0 += tc

    src = inp["edge_index"][0].astype(np.int64)
    dst = inp["edge_index"][1].astype(np.int64)
    srca = inp["edge_index_aux"][0].astype(np.int64)
    dsta = inp["edge_index_aux"][1].astype(np.int64)

    deg = np.zeros(n, np.float32)
    np.add.at(deg, dst, 1.0)
    recip_deg = (1.0 / np.maximum(deg, 1.0)).astype(np.float32)
    dega = np.zeros(n, np.float32)
    np.add.at(dega, dsta, 1.0)
    deg_hat = dega + 1.0
    rs = (1.0 / np.sqrt(deg_hat)).astype(np.float32)

    es_s_outs, es_s = _edge_streams(src, dst, recip_deg[dst], n, shard, nt,
                                    groups)
    allnodes = np.arange(n, dtype=np.int64)
    srca_x = np.concatenate([srca, allnodes])
    dsta_x = np.concatenate([dsta, allnodes])
    es_g_outs, es_g = _edge_streams(srca_x, dsta_x,
                                    rs[srca_x] * rs[dsta_x], n, shard, nt,
                                    groups)

    # feature-major global x (for replicated layer-1 table build):
    # xtg[f, i*P + j] = x[i*P + j, f]
    xpad = np.zeros((npad, P), np.float32)
    xpad[:n] = x
    xtg = np.ascontiguousarray(xpad.T).astype(bf16)  # [128, npad]

    # packed bf16 weights [P, 16*P]
    def w2(a):
        a = np.asarray(a, np.float32)
        return [a[i * P:(i + 1) * P] for i in range(a.shape[0] // P)]

    wb_tiles = []
    wb_off = {}

    def put_b(name, tiles):
        wb_off[name] = len(wb_tiles) * P
        wb_tiles.extend(tiles)

    put_b("fc1", w2(inp["fc1_W"]))
    for l in (1, 2, 3):
        put_b(f"sWl{l}", w2(inp[f"s{l}_Wl"]))
        put_b(f"gW{l}", w2(inp[f"g{l}_W"]))
        put_b(f"sWr{l}", w2(inp[f"s{l}_Wr"]))
    wb = np.concatenate(wb_tiles, axis=1).astype(bf16)  # [128, 16*128]

    # fp32 consts [128, 1]: fc1 bias col
    wf = np.asarray(inp["fc1_b"], np.float32).reshape(P, 1).copy()

    # bias rows [1, 7*128] bf16: ones, bl1, gb1, bl2, gb2, bl3, gb3
    br_cols = [np.ones(P, np.float32)]
    for l in (1, 2, 3):
        br_cols.append(np.asarray(inp[f"s{l}_bl"], np.float32).reshape(-1))
        br_cols.append(np.asarray(inp[f"g{l}_b"], np.float32).reshape(-1))
    br = np.concatenate(br_cols).reshape(1, -1).astype(bf16)  # [1, 896]

    # head cols [128, 7] bf16 (w_i folded): h1, h2a, h2b, h3a, h3b, h4a, h4b
    w_scal = [float(inp[f"w{i}"][0]) for i in range(1, 5)]
    hcols = [np.asarray(inp["l1_W"], np.float32).reshape(-1) * w_scal[0]]
    for i, l in ((1, 2), (2, 3), (3, 4)):
        hw = np.asarray(inp[f"l{l}_W"], np.float32).reshape(-1) * w_scal[i]
        hcols.append(hw[:P])
        hcols.append(hw[P:])
    hb = np.stack(hcols, axis=1).astype(bf16)  # [128, 7]
    total_bias = float(sum(float(inp[f"l{i}_b"][0]) * w_scal[i - 1]
                           for i in range(1, 5)))

    meta = dict(n=n, shard=shard, nt=nt, ntx=ntx, npad=npad,
                groups=groups, es_s=es_s, es_g=es_g,
                wb_off=wb_off, total_bias=total_bias)

    in_maps = []
    for c in range(NCORES):
        lo = c * shard
        nown = min(shard, n - lo)
        ownx = np.zeros((nt * P, P), np.float32)
        ownx[:nown] = x[lo:lo + nown]
        xto = np.ascontiguousarray(ownx.T).astype(bf16)  # [128, nt*P]
        rso = np.ones(nt * P, np.float32)
        rso[:nown] = rs[lo:lo + nown]
        idx_s, m_s = es_s_outs[c]
        idx_g, m_g = es_g_outs[c]
        in_maps.append({
            "xtg": xtg, "xto": xto,
            "idxs": idx_s, "ms": m_s,
            "idxg": idx_g, "mg": m_g,
            "wb": wb, "wf": wf, "br": br, "hb": hb,
        })
    return meta, in_maps


# ----------------------------------------------------------------------------
# device program
# ----------------------------------------------------------------------------

def _build(meta):
    import contextlib
    import concourse.bacc as bacc
    import concourse.mybir as mybir
    import concourse.tile as tile

    dt = mybir.dt
    Alu = mybir.AluOpType
    Act = mybir.ActivationFunctionType

    n, shard, nt, ntx, npad = (meta[k] for k in ("n", "shard", "nt", "ntx",
                                                 "npad"))
    groups = meta["groups"]
    es = {"s": meta["es_s"], "g": meta["es_g"]}
    wbo = meta["wb_off"]
    f32, b16, f8 = dt.float32, dt.bfloat16, dt.float8e4

    nc = bacc.Bacc("TRN2", target_bir_lowering=False, debug=False,
                   num_devices=NCORES)

    def din(name, shape, dtype):
        return nc.dram_tensor(name, shape, dtype, kind="ExternalInput")

    xtg_d = din("xtg", [P, npad], b16)
    xto_d = din("xto", [P, nt * P], b16)
    idx_d = {"s": din("idxs", [P, es["s"]["SI"]], dt.int16),
             "g": din("idxg", [P, es["g"]["SI"]], dt.int16)}
    m_d = {"s": din("ms", [P, es["s"]["SMW"]], b16),
           "g": din("mg", [P, es["g"]["SMW"]], b16)}
    wb_d = din("wb", [P, 16 * P], b16)
    wf_d = din("wf", [P, 1], f32)
    br_d = din("br", [1, 7 * P], b16)
    hb_d = din("hb", [P, 7], b16)
    res_d = nc.dram_tensor("res", [P, nt], f32, kind="ExternalOutput")

    def cell(kind, t, r):
        e = es[kind]
        return (int(e["K"][t][r]), int(e["cid0"][t][r]))

    with tile.TileContext(nc) as tc:
        _stack = contextlib.ExitStack()
        ppool = _stack.enter_context(tc.tile_pool(name="persist", bufs=1))
        dpool = _stack.enter_context(
            tc.tile_pool(name="persistd", bufs=1, space="DRAM"))

        # --- persistent SBUF ---
        x0T = ppool.tile([P, nt * P], f32, tag="x0T", name="x0T")
        x1aT = ppool.tile([P, nt * P], f32, tag="x1aT", name="x1aT")
        x1bT = ppool.tile([P, nt * P], f32, tag="x1bT", name="x1bT")
        xcb = ppool.tile([P, nt * P], b16, tag="xcb", name="xcb")
        xab = ppool.tile([P, nt * P], b16, tag="xab", name="xab")
        wb_s = ppool.tile([P, 16 * P], b16, tag="wb_s", name="wb_s")
        wf_s = ppool.tile([P, 1], f32, tag="wf_s", name="wf_s")
        br_s = ppool.tile([1, 7 * P], b16, tag="br_s", name="br_s")
        hb_s = ppool.tile([P, 7], b16, tag="hb_s", name="hb_s")
        resb = ppool.tile([P, nt], f32, tag="resb", name="resb")

        # --- DRAM tables (fp8, full 256B rows) ---
        tbl1 = dpool.tile([npad, 2 * P], b16, tag="tbl1", name="tbl1")
        tbl2 = dpool.tile([n, 2 * P], b16, tag="tbl2", name="tbl2",
                          addr_space="Shared")
        tbl3 = dpool.tile([n, 2 * P], b16, tag="tbl3", name="tbl3",
                          addr_space="Shared")
        sh2 = dpool.tile([nt * P, 2 * P], b16, tag="sh2", name="sh2")
        sh3 = dpool.tile([nt * P, 2 * P], b16, tag="sh3", name="sh3")
        tbls = (tbl1, tbl2, tbl3)

        for t_, d_ in ((wb_s, wb_d), (wf_s, wf_d), (br_s, br_d),
                       (hb_s, hb_d)):
            nc.sync.dma_start(out=t_[:], in_=d_[:])

        nc.vector.memset(resb[:], 0.0)

        def wbt(name, half=0):
            o = wbo[name] + half * P
            return wb_s[:, o:o + P]

        def brow(i):  # bias row [1, 128]
            return br_s[:, i * P:(i + 1) * P]

        BGT = 7  # tiles per xtg load batch in phase 1

        with (
            tc.tile_pool(name="bp", bufs=4) as bp,
            tc.tile_pool(name="tp", bufs=2) as tp,
            tc.tile_pool(name="pp", bufs=2, space="PSUM") as pp,
            tc.tile_pool(name="pq", bufs=4, space="PSUM") as pq,
        ):
            # ---- phase 1: replicated tbl1 build (2-tile psum batches) ----
            with tc.tile_pool(name="xp", bufs=2) as xp:
                alt = 0
                for j0 in range(0, ntx, BGT):
                    bgt = min(BGT, ntx - j0)
                    xt_t = xp.tile([P, bgt * P], b16, tag="xt")
                    nc.sync.dma_start(out=xt_t[:],
                                      in_=xtg_d[:, j0 * P:(j0 + bgt) * P])
                    tb = tp.tile([P, bgt * 2 * P], b16, tag="tb")
                    for jj in range(0, bgt, 2):
                        nb = min(2, bgt - jj)
                        p1 = pq.tile([P, nb * P], f32, tag="pa")
                        for q in range(nb):
                            nc.tensor.matmul(
                                p1[:, q * P:(q + 1) * P], lhsT=wbt("fc1"),
                                rhs=xt_t[:, (jj + q) * P:(jj + q + 1) * P],
                                start=True, stop=True)
                        o1 = bp.tile([P, nb * P], b16, tag="o1")
                        alt += 1
                        if alt % 2 == 0:
                            nc.scalar.activation(o1[:], p1[:], Act.Relu,
                                                 bias=wf_s[:, 0:1])
                        else:
                            nc.vector.tensor_scalar(o1[:], p1[:],
                                                    wf_s[:, 0:1], 0.0,
                                                    op0=Alu.add, op1=Alu.max)
                        pt = pp.tile([P, nb * 2 * P], f32, tag="pt")
                        for q in range(nb):
                            oq = o1[:, q * P:(q + 1) * P]
                            nc.tensor.matmul(
                                pt[:, q * 2 * P:q * 2 * P + P],
                                lhsT=oq, rhs=wbt("sWl1"),
                                start=True, stop=True)
                            nc.tensor.matmul(
                                pt[:, q * 2 * P + P:(q + 1) * 2 * P],
                                lhsT=oq, rhs=wbt("gW1"),
                                start=True, stop=True)
                        dst8 = tb[:, jj * 2 * P:(jj + nb) * 2 * P]
                        if alt % 2 == 0:
                            nc.scalar.activation(dst8, pt[:], Act.Copy)
                        else:
                            nc.vector.tensor_copy(dst8, pt[:])
                    nc.sync.dma_start(
                        out=tbl1[j0 * P:(j0 + bgt) * P, :]
                        .rearrange("(jj n) c -> n jj c", n=P),
                        in_=tb[:])

                # ---- phase 2: own-shard fc1 -> x0T, xcb(=bf16 x0) ----
                xo_t = xp.tile([P, nt * P], b16, tag="xo", bufs=1)
                nc.sync.dma_start(out=xo_t[:], in_=xto_d[:])
                for t in range(0, nt, 2):
                    nb = min(2, nt - t)
                    sl = slice(t * P, (t + nb) * P)
                    p1 = pq.tile([P, nb * P], f32, tag="pa")
                    for q in range(nb):
                        nc.tensor.matmul(
                            p1[:, q * P:(q + 1) * P], lhsT=wbt("fc1"),
                            rhs=xo_t[:, (t + q) * P:(t + q + 1) * P],
                            start=True, stop=True)
                    nc.scalar.activation(x0T[:, sl], p1[:], Act.Relu,
                                         bias=wf_s[:, 0:1])
                    nc.vector.tensor_copy(xcb[:, sl], x0T[:, sl])

            # ---- conv layers ----
            with (
                tc.tile_pool(name="gp", bufs=2) as gp,
                tc.tile_pool(name="mp", bufs=2) as mp,
                tc.tile_pool(name="ip", bufs=2) as ip,
            ):
                for l in (1, 2, 3):
                    tbl = tbls[l - 1]
                    sh_next = (sh2, sh3, None)[l - 1]
                    tbl_next = (tbl2, tbl3, None)[l - 1]
                    ln = l + 1
                    for (t0, tc_) in groups:
                        gt = {}
                        mt = {}
                        for kind in ("s", "g"):
                            e = es[kind]
                            ioffA = e["ioff"][f"{t0}_0"]
                            ioffB = e["ioff"][f"{t0}_1"]
                            KA = sum(cell(kind, t, 0)[0]
                                     for t in range(t0, t0 + tc_))
                            KB = sum(cell(kind, t, 1)[0]
                                     for t in range(t0, t0 + tc_))
                            icols = (KA + KB) * 8
                            it = ip.tile([P, icols], dt.int16, tag="it" + kind)
                            nc.sync.dma_start(
                                out=it[:],
                                in_=idx_d[kind][:, ioffA:ioffA + icols])
                            c0 = cell(kind, t0, 0)[1]
                            m0 = e["mco"][c0]
                            if t0 + tc_ < nt:
                                m1 = e["mco"][cell(kind, t0 + tc_, 0)[1]]
                            else:
                                m1 = e["SMW"]
                            mtile = mp.tile([P, m1 - m0], b16, tag="m" + kind)
                            nc.sync.dma_start(out=mtile[:],
                                              in_=m_d[kind][:, m0:m1])
                            mt[kind] = (mtile, m0)
                            colofs = 0 if kind == "s" else P
                            CAPK = 5  # max chunks (640 rows, HW-proven) per dma_gather
                            for r, KR, io in ((0, KA, ioffA), (1, KB, ioffB)):
                                if KR == 0:
                                    gt[(kind, r)] = None
                                    continue
                                g = gp.tile([P, KR * P], b16,
                                            tag=f"g{r}")
                                rlo = r * REG
                                rhi = min(n, rlo + REG)
                                for s in range(0, KR, CAPK):
                                    kk = min(CAPK, KR - s)
                                    nc.gpsimd.dma_gather(
                                        out_ap=g[:, s * P:(s + kk) * P]
                                        .rearrange("p (k e) -> p k e", e=P),
                                        in_ap=tbl[rlo:rhi,
                                                  colofs:colofs + P],
                                        idxs_ap=it[:, io - ioffA + s * 8:
                                                   io - ioffA + (s + kk) * 8],
                                        num_idxs=kk * P,
                                        num_idxs_reg=kk * P,
                                        elem_size=P,
                                        elem_step=2 * P)
                                gt[(kind, r)] = g

                        tbb = None
                        if l < 3:
                            tbb = tp.tile([P, tc_ * 2 * P], b16, tag="tb")
                        for tt in range(tc_):
                            t = t0 + tt
                            sl = slice(t * P, (t + 1) * P)
                            # head on out_l's first term (x0 head) must read
                            # xcb BEFORE this tile's finish overwrites it
                            ph = pq.tile([P, 1], f32, tag="ph", bufs=1)
                            if l == 1:
                                nc.tensor.matmul(ph[:], lhsT=xcb[:, sl],
                                                 rhs=hb_s[:, 0:1],
                                                 start=True, stop=False)
                            for kind in ("s", "g"):
                                e = es[kind]
                                colofs = 0 if kind == "s" else P
                                pa = pq.tile([P, P], f32, tag="pa")
                                bi = (2 * l - 1) if kind == "s" else (2 * l)
                                nc.tensor.matmul(pa[:], lhsT=brow(bi),
                                                 rhs=brow(0), start=True,
                                                 stop=False)
                                if kind == "s":
                                    if l == 1:
                                        nc.tensor.matmul(
                                            pa[:], lhsT=wbt("sWr1"),
                                            rhs=xcb[:, sl],
                                            start=False, stop=False)
                                    else:
                                        nc.tensor.matmul(
                                            pa[:], lhsT=wbt(f"sWr{l}", 0),
                                            rhs=xcb[:, sl],
                                            start=False, stop=False)
                                        nc.tensor.matmul(
                                            pa[:], lhsT=wbt(f"sWr{l}", 1),
                                            rhs=xab[:, sl],
                                            start=False, stop=False)
                                mtile, m0 = mt[kind]
                                chunks = []
                                for r in range(2):
                                    KR, cid_0 = cell(kind, t, r)
                                    gofs = sum(cell(kind, q, r)[0]
                                               for q in range(t0, t))
                                    for k in range(KR):
                                        chunks.append((r, gofs + k,
                                                       cid_0 + k))
                                for ci, (r, gk, cid) in enumerate(chunks):
                                    g = gt[(kind, r)]
                                    lo_c = e["wlo"][cid]
                                    w_c = e["W"][cid]
                                    mc = e["mco"][cid] - m0
                                    last = ci == len(chunks) - 1
                                    nc.tensor.matmul(
                                        pa[:, lo_c:lo_c + w_c],
                                        lhsT=g[:, gk * P:(gk + 1) * P],
                                        rhs=mtile[:, mc:mc + w_c],
                                        start=False, stop=last)
                                # finish
                                if l == 1:
                                    xT = x1aT if kind == "s" else x1bT
                                    xb = xcb if kind == "s" else xab
                                    nc.vector.tensor_tensor(
                                        out=xT[:, sl], in0=pa[:],
                                        in1=x0T[:, sl], op=Alu.add)
                                    if kind == "s":
                                        nc.vector.tensor_copy(xb[:, sl],
                                                              xT[:, sl])
                                    else:
                                        nc.scalar.activation(xb[:, sl],
                                                             xT[:, sl],
                                                             Act.Copy)
                                else:
                                    x01 = x1aT if kind == "s" else x1bT
                                    xb = xcb if kind == "s" else xab
                                    nc.vector.tensor_tensor(
                                        out=xb[:, sl], in0=pa[:],
                                        in1=x01[:, sl], op=Alu.add)
                            # heads on out_{l+1}
                            hia = 2 * l - 1
                            nc.tensor.matmul(ph[:], lhsT=xcb[:, sl],
                                             rhs=hb_s[:, hia:hia + 1],
                                             start=(l != 1), stop=False)
                            nc.tensor.matmul(ph[:], lhsT=xab[:, sl],
                                             rhs=hb_s[:, hia + 1:hia + 2],
                                             start=False, stop=True)
                            nc.vector.tensor_tensor(
                                out=resb[:, t:t + 1], in0=resb[:, t:t + 1],
                                in1=ph[:], op=Alu.add)
                            # next-layer table rows (batched psum)
                            if l < 3:
                                pt = pp.tile([P, 2 * P], f32, tag="pt")
                                nc.tensor.matmul(pt[:, 0:P],
                                                 lhsT=xcb[:, sl],
                                                 rhs=wbt(f"sWl{ln}", 0),
                                                 start=True, stop=False)
                                nc.tensor.matmul(pt[:, 0:P],
                                                 lhsT=xab[:, sl],
                                                 rhs=wbt(f"sWl{ln}", 1),
                                                 start=False, stop=True)
                                nc.tensor.matmul(pt[:, P:2 * P],
                                                 lhsT=xcb[:, sl],
                                                 rhs=wbt(f"gW{ln}", 0),
                                                 start=True, stop=False)
                                nc.tensor.matmul(pt[:, P:2 * P],
                                                 lhsT=xab[:, sl],
                                                 rhs=wbt(f"gW{ln}", 1),
                                                 start=False, stop=True)
                                dst8 = tbb[:, tt * 2 * P:(tt + 1) * 2 * P]
                                if tt % 2 == 0:
                                    nc.vector.tensor_copy(dst8, pt[:])
                                else:
                                    nc.scalar.activation(dst8, pt[:],
                                                         Act.Copy)
                        if l < 3:
                            nc.scalar.dma_start(
                                out=sh_next[t0 * P:(t0 + tc_) * P, :]
                                .rearrange("(jj n) c -> n jj c", n=P),
                                in_=tbb[:])
                    if l < 3:
                        nc.gpsimd.collective_compute(
                            "AllGather", mybir.AluOpType.bypass,
                            replica_groups=[list(range(NCORES))],
                            ins=[sh_next[0:shard, :]], outs=[tbl_next[:]])
                    if l == 1:
                        nc.vector.tensor_tensor(out=x1aT[:], in0=x1aT[:],
                                                in1=x0T[:], op=Alu.add)
                        nc.vector.tensor_tensor(out=x1bT[:], in0=x1bT[:],
                                                in1=x0T[:], op=Alu.add)

            # ---- output ----
            nc.vector.tensor_scalar(resb[:], resb[:],
                                    float(meta["total_bias"]), None,
                                    op0=Alu.add)
            nc.scalar.dma_start(out=res_d[:], in_=resb[:])
        _stack.close()

    nc.compile()
    return nc


def _run_and_bench(nc, in_maps, iters):
    import time
    import jax
    import numpy as np
    from jax.sharding import Mesh, PartitionSpec
    from jax.experimental.shard_map import shard_map
    import concourse.mybir as mybir
    from concourse import bass2jax

    bass2jax.install_neuronx_cc_hook()
    partition_name = (nc.partition_id_tensor.name
                      if nc.partition_id_tensor else None)
    in_names, out_names, out_avals, zero_outs = [], [], [], []
    for alloc in nc.m.functions[0].allocations:
        if not isinstance(alloc, mybir.MemoryLocationSet):
            continue
        name = alloc.memorylocations[0].name
        if alloc.kind == "ExternalInput":
            if name != partition_name:
                in_names.append(name)
        elif alloc.kind == "ExternalOutput":
            shape = tuple(alloc.tensor_shape)
            dtype = mybir.dt.np(alloc.dtype)
            out_names.append(name)
            out_avals.append(jax.core.ShapedArray(shape, dtype))
            zero_outs.append(np.zeros(shape, dtype))
    n_params = len(in_names)
    all_in_names = list(in_names) + out_names
    if partition_name is not None:
        all_in_names.append(partition_name)

    def _body(*args):
        operands = list(args)
        if partition_name is not None:
            operands.append(bass2jax.partition_id_tensor())
        outs = bass2jax._bass_exec_p.bind(
            *operands, out_avals=tuple(out_avals),
            in_names=tuple(all_in_names), out_names=tuple(out_names),
            lowering_input_output_aliases=(),
            sim_require_finite=True, sim_require_nnan=True, nc=nc)
        return tuple(outs)

    devices = jax.devices()[:NCORES]
    mesh = Mesh(np.asarray(devices), ("core",))
    in_specs = (PartitionSpec("core"),) * (n_params + len(out_names))
    out_specs = (PartitionSpec("core"),) * len(out_names)
    sharded = jax.jit(shard_map(_body, mesh=mesh, in_specs=in_specs,
                                out_specs=out_specs, check_rep=False),
                      keep_unused=True)
    concat_in = [
        np.concatenate([np.asarray(in_maps[c][nm]) for c in range(NCORES)], 0)
        for nm in in_names]
    concat_zeros = [np.zeros((NCORES * z.shape[0], *z.shape[1:]), z.dtype)
                    for z in zero_outs]
    out_arrs = sharded(*concat_in, *concat_zeros)
    jax.block_until_ready(out_arrs)

    per_exec_ns = None
    if iters > 0:
        from jax.sharding import NamedSharding
        dev_in = [jax.device_put(a, NamedSharding(mesh, PartitionSpec("core")))
                  for a in concat_in]
        dev_zero = [jax.device_put(z, NamedSharding(mesh, PartitionSpec("core")))
                    for z in concat_zeros]
        r = sharded(*dev_in, *dev_zero)
        jax.block_until_ready(r)
        t1 = time.perf_counter()
        rs = [sharded(*dev_in, *dev_zero) for _ in range(iters)]
        jax.block_until_ready(rs)
        t2 = time.perf_counter()
        per_exec_ns = (t2 - t1) / iters * 1e9

    results = [
        {nm: np.asarray(out_arrs[i]).reshape(NCORES, *out_avals[i].shape)[c]
         for i, nm in enumerate(out_names)}
        for c in range(NCORES)]
    return results, per_exec_ns


def kernel(**inputs):
    global LAST_EXEC_NS, LAST_TRACE

    meta, in_maps = _prep(inputs)
    nc = _build(meta)

    iters = int(os.environ.get("GNN_BENCH", "0"))
    results, per_exec_ns = _run_and_bench(nc, in_maps, iters)
    LAST_EXEC_NS = per_exec_ns
    LAST_TRACE = None

    n, shard, nt = meta["n"], meta["shard"], meta["nt"]
    out = np.empty((n, 1), np.float32)
    for c in range(NCORES):
        r = results[c]["res"]  # [128, nt]
        out[c * shard:(c + 1) * shard, 0] = r.T.reshape(-1)[:shard]
    return out
